# revision 1
# baseline (speedup 1.0000x reference)
"""GCMC (gnn_message_passing) Trainium2 Bass kernel, 8-core SPMD.

Strategy (hardcoded for the nn_GCMC_40870908789353 shapes):
- Core c owns users [c*6250,(c+1)*6250) and items [c*2500,(c+1)*2500), laid
  out locally as users at rows [0,6250), items at [6272,8772), block 8832.
- Dead-code elimination: scores only read x rows at user_nodes/item_nodes,
  so edges whose destination is unsampled (and word pairs whose item is
  unsampled) are dropped during CPU-side sharding. This is exact.
- The GCN aggregation is linear, so we aggregate normalized embeddings
  first and apply conv_weight after: agg = segsum(xn[src]) @ W.
- Per 128-edge chunk (dst-tile sorted): payload rows are fetched with one
  indirect DMA (128 offsets, one per partition) and accumulated into the
  dst tile with a one-hot matmul in PSUM. Pad slots use loc7=-1 (one-hot
  never matches -> adds zero).
- Word pairs: same scheme into 20 item tiles; the matmul rhs carries a
  ones column so item counts fall out of the same PSUM accumulation.
- Score pairs p belong to core p//1024; x2 rows are routed via AllToAll.
"""
import sys
for p in ("/opt/trn_rl_repo", "/root/.axon_site/_ro/trn_rl_repo"):
    if p not in sys.path:
        sys.path.insert(0, p)
import numpy as np

NC = 8
NUM_USER = 50000
NUM_ITEM = 20000
DIM = 64
WDIM = 128
UPC = 6250
IPC = 2500
UPAD = 6272
BLOCK = 8832
NT_N = 69            # node tiles per core
ITEM_TILE0 = 49
R_T = 2560
NT_W = 20            # item tiles per core
NROW = NC * BLOCK    # 70656 xn rows
B = 8192
BPC = 1024
CELL = 384
SW = NC * CELL // 128  # send gather chunks (24)
K_E = 16             # chunks per edge offset/onehot group
K_W = 8              # chunks per word offset/onehot group
SLOPE = 0.01
SAMPLE_FILTER = True

_CACHE = {}


# ---------------------------------------------------------------- CPU prep

def _node_owner_local(v):
    v = np.asarray(v)
    is_user = v < NUM_USER
    c_u = v // UPC
    l_u = v - c_u * UPC
    i = v - NUM_USER
    c_i = i // IPC
    l_i = UPAD + (i - c_i * IPC)
    return (np.where(is_user, c_u, c_i).astype(np.int64),
            np.where(is_user, l_u, l_i).astype(np.int64))


def _relab_perm():
    perm = np.full(NROW, -1, np.int64)
    for c in range(NC):
        perm[c * BLOCK: c * BLOCK + UPC] = np.arange(c * UPC, (c + 1) * UPC)
        perm[c * BLOCK + UPAD: c * BLOCK + UPAD + IPC] = (
            NUM_USER + np.arange(c * IPC, (c + 1) * IPC))
    return perm


def _chunk_schedule(rows_per_core, loc_per_core, n_tiles, K):
    counts = np.zeros((NC, n_tiles), np.int64)
    srt = []
    for c in range(NC):
        order = np.argsort(loc_per_core[c], kind="stable")
        r, l = rows_per_core[c][order], loc_per_core[c][order]
        srt.append((r, l))
        counts[c] = np.bincount(l >> 7, minlength=n_tiles)
    n_chunks = np.maximum(np.ceil(counts / 128).astype(np.int64).max(0), 1)
    NCH = int(n_chunks.sum())
    NCHp = int(np.ceil(NCH / K) * K)
    cpt = n_chunks.copy()
    cpt[-1] += NCHp - NCH
    offs = np.zeros((NC, NCHp, 128), np.int32)
    loc7 = np.full((NC, NCHp, 128), -1.0, np.float32)
    for c in range(NC):
        r, l = srt[c]
        tiles = l >> 7
        start = np.searchsorted(tiles, np.arange(n_tiles))
        end = np.searchsorted(tiles, np.arange(n_tiles), side="right")
        ch0 = 0
        for t in range(n_tiles):
            nt = int(n_chunks[t])
            cnt = end[t] - start[t]
            fo = np.zeros(nt * 128, np.int32)
            fl = np.full(nt * 128, -1.0, np.float32)
            fo[:cnt] = r[start[t]:end[t]]
            fl[:cnt] = (l[start[t]:end[t]] - t * 128).astype(np.float32)
            offs[c, ch0:ch0 + nt] = fo.reshape(nt, 128)
            loc7[c, ch0:ch0 + nt] = fl.reshape(nt, 128)
            ch0 += nt
    # group chunks into instruction tiles [NG, 128, K]
    NG = NCHp // K
    g_o = np.ascontiguousarray(offs.reshape(NC, NG, K, 128).transpose(0, 1, 3, 2))
    g_l = np.ascontiguousarray(loc7.reshape(NC, NG, K, 128).transpose(0, 1, 3, 2))
    return cpt, g_o, g_l


def _prep(inputs):
    edge_index = np.asarray(inputs["edge_index"])
    words_tensor = np.asarray(inputs["words_tensor"])
    user_nodes = np.asarray(inputs["user_nodes"]).astype(np.int64)
    item_nodes = np.asarray(inputs["item_nodes"]).astype(np.int64)

    src, dst = edge_index[0].astype(np.int64), edge_index[1].astype(np.int64)
    items_w = words_tensor[0].astype(np.int64)
    words_w = words_tensor[1].astype(np.int64)

    if SAMPLE_FILTER:
        samp = np.zeros(NUM_USER + NUM_ITEM, bool)
        samp[user_nodes] = True
        samp[item_nodes] = True
        keep = samp[dst]
        src, dst = src[keep], dst[keep]
        samp_i = np.zeros(NUM_ITEM, bool)
        ii = item_nodes - NUM_USER
        samp_i[ii[item_nodes >= NUM_USER]] = True
        keepw = samp_i[items_w]
        items_w, words_w = items_w[keepw], words_w[keepw]

    so, sl = _node_owner_local(src)
    do, dl = _node_owner_local(dst)
    grow = (so * BLOCK + sl)
    e_rows = [grow[do == c] for c in range(NC)]
    e_locs = [dl[do == c] for c in range(NC)]
    cpt_e, e_offs, e_loc7 = _chunk_schedule(e_rows, e_locs, NT_N, K_E)

    owner_w = items_w // IPC
    w_rows = [words_w[owner_w == c] for c in range(NC)]
    w_locs = [(items_w - owner_w * IPC)[owner_w == c] for c in range(NC)]
    cpt_w, w_offs, w_loc7 = _chunk_schedule(w_rows, w_locs, NT_W, K_W)

    # score routing
    uo, ul = _node_owner_local(user_nodes)
    io_, il = _node_owner_local(item_nodes)
    dest = np.arange(B) // BPC
    fill = np.zeros((NC, NC), np.int64)
    send_rows = np.zeros((NC, NC * CELL), np.int64)
    recv_pos_u = np.empty(B, np.int64)
    recv_pos_i = np.empty(B, np.int64)
    for p in range(B):
        d = dest[p]
        for kind, (s, l) in enumerate(((uo[p], ul[p]), (io_[p], il[p]))):
            slot = fill[s][d]
            assert slot < CELL, "a2a cell overflow"
            fill[s][d] += 1
            send_rows[s][d * CELL + slot] = l
            if kind == 0:
                recv_pos_u[p] = s * CELL + slot
            else:
                recv_pos_i[p] = s * CELL + slot
    send_offs = np.zeros((NC, 128, SW), np.int32)
    j = np.arange(NC * CELL)
    for c in range(NC):
        send_offs[c, j % 128, j // 128] = send_rows[c]
    recv_offs = np.zeros((NC, 128, 16), np.int32)
    q = np.arange(BPC)
    for c in range(NC):
        mine = slice(c * BPC, (c + 1) * BPC)
        recv_offs[c, q % 128, q // 128] = recv_pos_u[mine]
        recv_offs[c, q % 128, 8 + q // 128] = recv_pos_i[mine]

    # permuted embeddings + per-core v_feat
    perm = _relab_perm()
    id_relab = np.zeros((NROW, DIM), np.float32)
    v = perm >= 0
    id_relab[v] = np.asarray(inputs["id_embedding"], np.float32)[perm[v]]
    v_feat = np.asarray(inputs["v_feat"], np.float32)
    vf = np.zeros((NC, R_T, WDIM), np.float32)
    for c in range(NC):
        vf[c, :IPC] = v_feat[c * IPC:(c + 1) * IPC]

    return dict(cpt_e=cpt_e, e_offs=e_offs, e_loc7=e_loc7,
                cpt_w=cpt_w, w_offs=w_offs, w_loc7=w_loc7,
                send_offs=send_offs, recv_offs=recv_offs,
                id_relab=id_relab, vf=vf)


# ------------------------------------------------------------- bass program

def _build_program(cpt_e, cpt_w, NGE, NGW):
    from concourse import bass, bacc, mybir
    import concourse.tile as tile
    dt = mybir.dt

    nc = bacc.Bacc(None, target_bir_lowering=False)
    f32 = dt.float32

    id_in = nc.dram_tensor("id_relab", [NROW, DIM], f32, kind="ExternalInput")
    wt_in = nc.dram_tensor("word_table", [100000, WDIM], f32, kind="ExternalInput")
    vf_in = nc.dram_tensor("vf", [R_T, WDIM], f32, kind="ExternalInput")
    eoff_in = nc.dram_tensor("e_offs", [NGE, 128, K_E], dt.int32, kind="ExternalInput")
    eloc_in = nc.dram_tensor("e_loc7", [NGE, 128, K_E], f32, kind="ExternalInput")
    woff_in = nc.dram_tensor("w_offs", [NGW, 128, K_W], dt.int32, kind="ExternalInput")
    wloc_in = nc.dram_tensor("w_loc7", [NGW, 128, K_W], f32, kind="ExternalInput")
    soff_in = nc.dram_tensor("send_offs", [128, SW], dt.int32, kind="ExternalInput")
    roff_in = nc.dram_tensor("recv_offs", [128, 16], dt.int32, kind="ExternalInput")
    cw_in = nc.dram_tensor("conv_weight", [DIM, DIM], f32, kind="ExternalInput")
    ww_in = nc.dram_tensor("weight_W", [DIM, DIM], f32, kind="ExternalInput")
    w2_in = nc.dram_tensor("weight_2", [DIM, DIM], f32, kind="ExternalInput")
    lw_in = nc.dram_tensor("lin_w", [256, DIM], f32, kind="ExternalInput")
    lb_in = nc.dram_tensor("lin_b_rep", [128, DIM], f32, kind="ExternalInput")
    iota_in = nc.dram_tensor("iota", [128, 128], f32, kind="ExternalInput")
    ident_in = nc.dram_tensor("ident", [128, 128], f32, kind="ExternalInput")

    xn_dram = nc.dram_tensor("xn", [NROW, DIM], f32)
    x2_dram = nc.dram_tensor("x2", [BLOCK, DIM], f32)
    out = nc.dram_tensor("scores_w", [128, 8], f32, kind="ExternalOutput")

    # edge chunk -> (group, col, tile, start, stop)
    def sched(cpt, K):
        s = []
        ch = 0
        for t, n in enumerate(cpt):
            for j in range(int(n)):
                s.append((ch // K, ch % K, t, j == 0, j == int(n) - 1))
                ch += 1
        return s

    esched = sched(cpt_e, K_E)
    wsched = sched(cpt_w, K_W)

    with tile.TileContext(nc) as tc:
        with tc.tile_pool(name="const", bufs=1) as cpool, \
             tc.tile_pool(name="persist", bufs=1) as pp, \
             tc.tile_pool(name="work", bufs=3) as wp, \
             tc.tile_pool(name="psum_e", bufs=2, space="PSUM") as pse, \
             tc.tile_pool(name="psum_w", bufs=2, space="PSUM") as psw, \
             tc.tile_pool(name="psum_m", bufs=2, space="PSUM") as psm, \
             tc.tile_pool(name="dram", bufs=1, space="DRAM") as dpool:

            iota = cpool.tile([128, 128], f32)
            ident = cpool.tile([128, 128], f32)
            cw = cpool.tile([DIM, DIM], f32)
            ww = cpool.tile([DIM, DIM], f32)
            w2 = cpool.tile([DIM, DIM], f32)
            lw = cpool.tile([128, 2 * DIM], f32)   # lin_w as two [128,64] halves
            lb = cpool.tile([128, DIM], f32)
            nc.sync.dma_start(out=iota[:], in_=iota_in[:])
            nc.sync.dma_start(out=ident[:], in_=ident_in[:])
            nc.sync.dma_start(out=cw[:], in_=cw_in[:])
            nc.sync.dma_start(out=ww[:], in_=ww_in[:])
            nc.sync.dma_start(out=w2[:], in_=w2_in[:])
            nc.sync.dma_start(out=lw[:, 0:DIM], in_=lw_in[0:128, :])
            nc.sync.dma_start(out=lw[:, DIM:2 * DIM], in_=lw_in[128:256, :])
            nc.sync.dma_start(out=lb[:], in_=lb_in[:])

            tf_sb = pp.tile([128, NT_W * WDIM], f32)
            fh_sb = pp.tile([128, NT_W * DIM], f32)
            pg_sb = pp.tile([128, NT_N * DIM], f32)
            x2_sb = pp.tile([128, NT_N * DIM], f32)

            # ---- phase N: normalize id_relab -> xn_dram (p-outer layout)
            NTT = NROW // 128          # 552 rows per partition
            NCHN = 8
            CH = NTT // NCHN           # 69 per chunk
            vi = id_in[:, :].rearrange("(p t) d -> p t d", p=128)
            vo = xn_dram[:, :].rearrange("(p t) d -> p t d", p=128)
            with tc.tile_pool(name="npool", bufs=1) as npool:
                for cch in range(NCHN):
                    x = npool.tile([128, CH * DIM], f32, tag="nx")
                    sq = npool.tile([128, CH * DIM], f32, tag="nsq")
                    ss = npool.tile([128, CH], f32, tag="nss")
                    x3 = x[:].rearrange("p (t d) -> p t d", d=DIM)
                    sq3 = sq[:].rearrange("p (t d) -> p t d", d=DIM)
                    nc.sync.dma_start(out=x3, in_=vi[:, cch * CH:(cch + 1) * CH, :])
                    nc.vector.tensor_tensor(out=sq3, in0=x3, in1=x3,
                                            op=mybir.AluOpType.mult)
                    nc.vector.reduce_sum(out=ss[:], in_=sq3,
                                         axis=mybir.AxisListType.X)
                    nc.scalar.sqrt(ss[:], ss[:])
                    nc.vector.tensor_scalar_max(out=ss[:], in0=ss[:], scalar1=1e-12)
                    nc.vector.reciprocal(ss[:], ss[:])
                    nc.vector.tensor_tensor(
                        out=x3, in0=x3,
                        in1=ss[:][:, :, None].to_broadcast([128, CH, DIM]),
                        op=mybir.AluOpType.mult)
                    nc.sync.dma_start(out=vo[:, cch * CH:(cch + 1) * CH, :], in_=x3)

            # ---- phase W: word aggregation into tf_sb
            wpsum = None
            for gi in range(NGW):
                woff = wp.tile([128, K_W], dt.int32, tag="woff")
                wloc = wp.tile([128, K_W], f32, tag="wloc")
                wpay = wp.tile([128, K_W * (WDIM + 1)], f32, tag="wpay")
                woh = wp.tile([128, K_W * 128], f32, tag="woh")
                nc.sync.dma_start(out=woff[:], in_=woff_in[gi])
                nc.sync.dma_start(out=wloc[:], in_=wloc_in[gi])
                pay3 = wpay[:].rearrange("p (k d) -> p k d", d=WDIM + 1)
                nc.vector.memset(pay3[:, :, WDIM:WDIM + 1], 1.0)
                oh3 = woh[:].rearrange("p (k d) -> p k d", d=128)
                nc.vector.tensor_tensor(
                    out=oh3,
                    in0=wloc[:][:, :, None].to_broadcast([128, K_W, 128]),
                    in1=iota[:][:, None, :].to_broadcast([128, K_W, 128]),
                    op=mybir.AluOpType.is_equal)
                for k in range(K_W):
                    ci = gi * K_W + k
                    if ci >= len(wsched):
                        break
                    _, _, t, st, sp = wsched[ci]
                    nc.gpsimd.indirect_dma_start(
                        out=pay3[:, k, 0:WDIM], out_offset=None,
                        in_=wt_in[:, :],
                        in_offset=bass.IndirectOffsetOnAxis(ap=woff[:, k:k + 1], axis=0))
                    if st:
                        wpsum = psw.tile([128, WDIM + 1], f32, tag="wps")
                    nc.tensor.matmul(out=wpsum[:], lhsT=oh3[:, k, :],
                                     rhs=pay3[:, k, :], start=st, stop=sp)
                    if sp:
                        rec = wp.tile([128, 1], f32, tag="wrec")
                        nc.vector.tensor_scalar_max(out=rec[:], in0=wpsum[:, WDIM:WDIM + 1], scalar1=1.0)
                        nc.vector.reciprocal(rec[:], rec[:])
                        nc.vector.tensor_tensor(
                            out=tf_sb[:, t * WDIM:(t + 1) * WDIM],
                            in0=wpsum[:, 0:WDIM],
                            in1=rec[:].to_broadcast([128, WDIM]),
                            op=mybir.AluOpType.mult)

            # ---- phase V: item pipeline -> fh_sb
            vf_sb = pp.tile([128, NT_W * WDIM], f32)
            nc.sync.dma_start(
                out=vf_sb[:].rearrange("p (t d) -> p t d", d=WDIM),
                in_=vf_in[:, :].rearrange("(t p) d -> p t d", p=128))
            for t in range(NT_W):
                ps_t = psm.tile([128, 128], f32, tag="tr")
                nc.tensor.transpose(out=ps_t[:], in_=vf_sb[:, t * WDIM:(t + 1) * WDIM],
                                    identity=ident[:])
                vT = wp.tile([128, 128], f32, tag="vT")
                nc.scalar.copy(out=vT[:], in_=ps_t[:])
                ps_t2 = psm.tile([128, 128], f32, tag="tr")
                nc.tensor.transpose(out=ps_t2[:], in_=tf_sb[:, t * WDIM:(t + 1) * WDIM],
                                    identity=ident[:])
                tT = wp.tile([128, 128], f32, tag="tT")
                nc.scalar.copy(out=tT[:], in_=ps_t2[:])
                fps = psm.tile([128, DIM], f32, tag="mm")
                nc.tensor.matmul(out=fps[:], lhsT=vT[:], rhs=lw[:, 0:DIM],
                                 start=True, stop=False)
                nc.tensor.matmul(out=fps[:], lhsT=tT[:], rhs=lw[:, DIM:2 * DIM],
                                 start=False, stop=True)
                fsum = wp.tile([128, DIM], f32, tag="fsum")
                nc.vector.tensor_add(out=fsum[:], in0=fps[:], in1=lb[:])
                f_sb = wp.tile([128, DIM], f32, tag="fsb")
                nc.scalar.activation(f_sb[:], fsum[:],
                                     mybir.ActivationFunctionType.Lrelu, alpha=SLOPE)
                ps_t3 = psm.tile([128, 128], f32, tag="tr")
                nc.tensor.transpose(out=ps_t3[0:64, :], in_=f_sb[:],
                                    identity=ident[:])
                fT = wp.tile([64, 128], f32, tag="fT")
                nc.scalar.copy(out=fT[:], in_=ps_t3[0:64, :])
                fhp = psm.tile([128, DIM], f32, tag="mm")
                nc.tensor.matmul(out=fhp[:], lhsT=fT[:], rhs=w2[:],
                                 start=True, stop=True)
                nc.scalar.copy(out=fh_sb[:, t * DIM:(t + 1) * DIM], in_=fhp[:])

            # ---- phase E: edge aggregation into pg_sb
            epsum = None
            for gi in range(NGE):
                eoff = wp.tile([128, K_E], dt.int32, tag="eoff")
                eloc = wp.tile([128, K_E], f32, tag="eloc")
                epay = wp.tile([128, K_E * DIM], f32, tag="epay")
                eoh = wp.tile([128, K_E * 128], f32, tag="eoh")
                nc.sync.dma_start(out=eoff[:], in_=eoff_in[gi])
                nc.sync.dma_start(out=eloc[:], in_=eloc_in[gi])
                pay3 = epay[:].rearrange("p (k d) -> p k d", d=DIM)
                oh3 = eoh[:].rearrange("p (k d) -> p k d", d=128)
                nc.vector.tensor_tensor(
                    out=oh3,
                    in0=eloc[:][:, :, None].to_broadcast([128, K_E, 128]),
                    in1=iota[:][:, None, :].to_broadcast([128, K_E, 128]),
                    op=mybir.AluOpType.is_equal)
                for k in range(K_E):
                    ci = gi * K_E + k
                    if ci >= len(esched):
                        break
                    _, _, t, st, sp = esched[ci]
                    nc.gpsimd.indirect_dma_start(
                        out=pay3[:, k, :], out_offset=None,
                        in_=xn_dram[:, :],
                        in_offset=bass.IndirectOffsetOnAxis(ap=eoff[:, k:k + 1], axis=0))
                    if st:
                        epsum = pse.tile([128, DIM], f32, tag="eps")
                    nc.tensor.matmul(out=epsum[:], lhsT=oh3[:, k, :],
                                     rhs=pay3[:, k, :], start=st, stop=sp)
                    if sp:
                        nc.scalar.copy(out=pg_sb[:, t * DIM:(t + 1) * DIM],
                                       in_=epsum[:])

            # ---- phase X: node tail -> x2_sb -> x2_dram
            for t in range(NT_N):
                ps_t = psm.tile([128, 128], f32, tag="tr")
                nc.tensor.transpose(out=ps_t[0:64, :],
                                    in_=pg_sb[:, t * DIM:(t + 1) * DIM],
                                    identity=ident[:])
                pgT = wp.tile([64, 128], f32, tag="pgT")
                nc.scalar.copy(out=pgT[:], in_=ps_t[0:64, :])
                x1p = psm.tile([128, DIM], f32, tag="mm")
                nc.tensor.matmul(out=x1p[:], lhsT=pgT[:], rhs=cw[:],
                                 start=True, stop=True)
                x1_sb = wp.tile([128, DIM], f32, tag="x1")
                nc.scalar.activation(x1_sb[:], x1p[:],
                                     mybir.ActivationFunctionType.Lrelu, alpha=SLOPE)
                ps_t2 = psm.tile([128, 128], f32, tag="tr")
                nc.tensor.transpose(out=ps_t2[0:64, :], in_=x1_sb[:],
                                    identity=ident[:])
                x1T = wp.tile([64, 128], f32, tag="x1T")
                nc.scalar.copy(out=x1T[:], in_=ps_t2[0:64, :])
                x2p = psm.tile([128, DIM], f32, tag="mm")
                nc.tensor.matmul(out=x2p[:], lhsT=x1T[:], rhs=ww[:],
                                 start=True, stop=True)
                if t >= ITEM_TILE0:
                    xsum = wp.tile([128, DIM], f32, tag="xsum")
                    nc.vector.tensor_add(
                        out=xsum[:], in0=x2p[:],
                        in1=fh_sb[:, (t - ITEM_TILE0) * DIM:(t - ITEM_TILE0 + 1) * DIM])
                    nc.scalar.activation(x2_sb[:, t * DIM:(t + 1) * DIM], xsum[:],
                                         mybir.ActivationFunctionType.Lrelu, alpha=SLOPE)
                else:
                    nc.scalar.activation(x2_sb[:, t * DIM:(t + 1) * DIM], x2p[:],
                                         mybir.ActivationFunctionType.Lrelu, alpha=SLOPE)
            nc.sync.dma_start(
                out=x2_dram[:, :].rearrange("(t p) d -> p t d", p=128),
                in_=x2_sb[:].rearrange("p (t d) -> p t d", d=DIM))

            # ---- phase S: score routing + dots
            soff = pp.tile([128, SW], dt.int32)
            roff = pp.tile([128, 16], dt.int32)
            nc.sync.dma_start(out=soff[:], in_=soff_in[:])
            nc.sync.dma_start(out=roff[:], in_=roff_in[:])
            send_sb = pp.tile([128, SW * DIM], f32)
            s3 = send_sb[:].rearrange("p (k d) -> p k d", d=DIM)
            for k in range(SW):
                nc.gpsimd.indirect_dma_start(
                    out=s3[:, k, :], out_offset=None, in_=x2_dram[:, :],
                    in_offset=bass.IndirectOffsetOnAxis(ap=soff[:, k:k + 1], axis=0))
            a2a_in = dpool.tile([NC, CELL * DIM], f32)
            a2a_out = dpool.tile([NC, CELL * DIM], f32)
            nc.sync.dma_start(
                out=a2a_in[:].rearrange("c (s p d) -> p (c s) d", p=128, d=DIM),
                in_=s3)
            nc.gpsimd.collective_compute(
                "AllToAll", mybir.AluOpType.bypass,
                replica_groups=[list(range(NC))],
                ins=[a2a_in.opt()], outs=[a2a_out.opt()])
            recv_flat = a2a_out[:].rearrange("c (r d) -> (c r) d", d=DIM)
            pairs = pp.tile([128, 16 * DIM], f32)
            p3 = pairs[:].rearrange("p (k d) -> p k d", d=DIM)
            for k in range(16):
                nc.gpsimd.indirect_dma_start(
                    out=p3[:, k, :], out_offset=None, in_=recv_flat,
                    in_offset=bass.IndirectOffsetOnAxis(ap=roff[:, k:k + 1], axis=0))
            prod = pp.tile([128, 8 * DIM], f32)
            pr3 = prod[:].rearrange("p (k d) -> p k d", d=DIM)
            nc.vector.tensor_tensor(out=pr3, in0=p3[:, 0:8, :], in1=p3[:, 8:16, :],
                                    op=mybir.AluOpType.mult)
            sc = pp.tile([128, 8], f32)
            nc.vector.reduce_sum(out=sc[:], in_=pr3, axis=mybir.AxisListType.X)
            nc.sync.dma_start(out=out[:], in_=sc[:])

    nc.finalize()
    return nc


# ------------------------------------------------------------------- kernel

def kernel(**inputs):
    from concourse.bass_utils import run_bass_kernel_spmd

    pr = _prep(inputs)
    NGE = pr["e_offs"].shape[1]
    NGW = pr["w_offs"].shape[1]
    key = (tuple(pr["cpt_e"]), tuple(pr["cpt_w"]), NGE, NGW)
    if key not in _CACHE:
        _CACHE[key] = _build_program(pr["cpt_e"], pr["cpt_w"], NGE, NGW)
    nc = _CACHE[key]

    iota = np.broadcast_to(np.arange(128, dtype=np.float32), (128, 128)).copy()
    ident = np.eye(128, dtype=np.float32)
    lin_b_rep = np.broadcast_to(np.asarray(inputs["lin_b"], np.float32), (128, DIM)).copy()

    in_maps = []
    for c in range(NC):
        in_maps.append({
            "id_relab": pr["id_relab"],
            "word_table": np.asarray(inputs["word_table"], np.float32),
            "vf": pr["vf"][c],
            "e_offs": pr["e_offs"][c],
            "e_loc7": pr["e_loc7"][c],
            "w_offs": pr["w_offs"][c],
            "w_loc7": pr["w_loc7"][c],
            "send_offs": pr["send_offs"][c],
            "recv_offs": pr["recv_offs"][c],
            "conv_weight": np.asarray(inputs["conv_weight"], np.float32),
            "weight_W": np.asarray(inputs["weight_W"], np.float32),
            "weight_2": np.asarray(inputs["weight_2"], np.float32),
            "lin_w": np.asarray(inputs["lin_w"], np.float32),
            "lin_b_rep": lin_b_rep,
            "iota": iota,
            "ident": ident,
        })
    res = run_bass_kernel_spmd(nc, in_maps, list(range(NC)))
    scores = np.empty(B, np.float32)
    for c in range(NC):
        w = res.results[c]["scores_w"]           # [128, 8]
        scores[c * BPC:(c + 1) * BPC] = w.T.ravel()
    return scores


kernel.run_traced = None  # set by test harness if needed



# revision 16
# speedup vs baseline: 1.3353x; 1.3353x over previous
"""GCMC (gnn_message_passing) Trainium2 Bass kernel, 8-core SPMD, v2.

Strategy (hardcoded for the nn_GCMC_40870908789353 shapes):
- Score-pair sharding: core c owns pairs [1024c, 1024(c+1)). Its 2048 node
  slots (1024 user + 1024 item, duplicates allowed) are the only rows for
  which agg/x2 are computed, so no collective is needed anywhere.
- Gathers use batched dma_gather (SWDGE, int16 idxs) instead of per-chunk
  indirect DMAs: ~30 instructions/core instead of ~850. Tables are split
  into <=32767-row regions (id_emb 3, word_table 4) to fit int16 indices.
- Transposed dataflow: segment-sum one-hot matmuls run as lhsT=payload,
  rhs=one-hot so PSUM holds agg^T / t_feat^T ([dim, slot]); every later
  matmul chains without a single PE transpose. lin_b rides the ACT bias
  port, x1@W and f@w2 accumulate into the same PSUM tile.
- Edge payload rows are L2-normalized on the fly (square/reduce/rsqrt on
  DVE) which folds F.normalize into the gather and kills the full-table
  normalize pass; the scale-mult also casts the payload to bf16.
- Word payload rows are pre-scaled by 1/deg(item slot) (host metadata), so
  the PSUM directly accumulates the mean.
"""
import sys
for p in ("/opt/trn_rl_repo", "/root/.axon_site/_ro/trn_rl_repo"):
    if p not in sys.path:
        sys.path.insert(0, p)
import numpy as np
import ml_dtypes

NC = 8
NUM_USER = 50000
NUM_ITEM = 20000
NNODE = 70000
VOCAB = 100000
DIM = 64
WDIM = 128
B = 8192
BPC = 1024          # pairs per core
NSLOT = 2048        # node slots per core (1024 user + 1024 item)
NT = 16             # node slot tiles
IT = 8              # item slot tiles
REG_E = 23552       # id_emb region rows (3 regions)
NREG_E = 3
REG_W = 25000       # word_table region rows (4 regions)
NREG_W = 4
EB = 48             # edge chunks per dma_gather batch
WB = 32             # word chunks per dma_gather batch
SLOPE = 0.01

_CACHE = {}

bf16 = ml_dtypes.bfloat16


# ---------------------------------------------------------------- CPU prep

def _ragged_gather(starts, lens):
    """positions [starts[i], starts[i]+lens[i]) concatenated."""
    tot = int(lens.sum())
    if tot == 0:
        return np.zeros(0, np.int64)
    cum = np.cumsum(lens) - lens
    return np.repeat(starts - cum, lens) + np.arange(tot)


def _build_stream(slot_rep, val_rep, region_rep, n_tiles, n_reg, extra=None):
    """Per-core stream fill given the instance list (slot, table-local idx,
    region). Returns dict with per-(region,tile) counts and a fill closure.
    """
    key = region_rep * n_tiles + (slot_rep >> 7)
    order = np.argsort(key, kind="stable")
    return order, key[order]


class _Sched:
    """Unified SPMD schedule for one gather family."""

    def __init__(self, cnt, n_tiles, n_reg, batch):
        # cnt: [NC, n_reg, n_tiles] instance counts
        nch = np.ceil(cnt / 128.0).astype(np.int64).max(axis=0)  # [n_reg,n_tiles]
        # every tile needs >=1 chunk overall so start/stop exist
        tile_tot = nch.sum(axis=0)
        for t in range(n_tiles):
            if tile_tot[t] == 0:
                nch[0][t] = 1
        self.nch = nch
        self.n_tiles = n_tiles
        self.n_reg = n_reg
        # global chunk order: region-major, tile-minor
        tiles = []
        regions = []
        for r in range(n_reg):
            for t in range(n_tiles):
                tiles += [t] * int(nch[r][t])
                regions += [r] * int(nch[r][t])
        self.tile_of = np.array(tiles, np.int64)
        self.region_of = np.array(regions, np.int64)
        self.NCH = len(tiles)
        self.S = self.NCH * 128
        # start/stop flags per chunk at (region, tile) GROUP granularity: each
        # group is one PSUM accumulation (own bank) closed within its region.
        self.is_first = []
        self.is_last = []
        for ch in range(len(tiles)):
            r, t = regions[ch], tiles[ch]
            self.is_first.append(ch == 0 or (regions[ch - 1], tiles[ch - 1]) != (r, t))
            self.is_last.append(ch == len(tiles) - 1
                                or (regions[ch + 1], tiles[ch + 1]) != (r, t))
        # group (r,t) -> starting chunk
        self.group_ch0 = np.zeros((n_reg, n_tiles), np.int64)
        ch = 0
        for r in range(n_reg):
            for t in range(n_tiles):
                self.group_ch0[r][t] = ch
                ch += int(nch[r][t])
        # batches: split each region's chunk range into <= batch chunks
        self.batches = []  # (region, ch0, nchunks)
        for r in range(n_reg):
            r0 = int(self.group_ch0[r][0])
            r1 = int(self.group_ch0[r + 1][0]) if r + 1 < n_reg else self.NCH
            ch = r0
            while ch < r1:
                nb = min(batch, r1 - ch)
                self.batches.append((r, ch, nb))
                ch += nb

    def key(self):
        return (self.n_tiles, self.n_reg) + tuple(self.nch.ravel().tolist())


def _fill_stream(sched, slot_rep, loc_val, region_rep, scale=None):
    """Place instances into the padded stream. Returns (idx_stream int16,
    loc_stream bf16, scale_stream bf16 or None)."""
    n_tiles = sched.n_tiles
    key = region_rep * n_tiles + (slot_rep >> 7)
    order = np.argsort(key, kind="stable")
    skey = key[order]
    gcnt = np.bincount(skey, minlength=sched.n_reg * n_tiles)
    # position of each sorted instance: group base*128 + within-group offset
    ch0 = sched.group_ch0.ravel()
    base = np.repeat(ch0 * 128, gcnt)
    within = np.arange(len(order)) - np.repeat(np.cumsum(gcnt) - gcnt, gcnt)
    pos = base + within
    idx_stream = np.zeros(sched.S, np.int16)
    idx_stream[pos] = loc_val[order].astype(np.int16)
    loc_stream = np.full(sched.S, -1.0, bf16)
    loc_stream[pos] = (slot_rep[order] & 127).astype(bf16)
    sc_stream = None
    if scale is not None:
        sc_stream = np.zeros(sched.S, bf16)
        sc_stream[pos] = scale[order].astype(bf16)
    return idx_stream, loc_stream, sc_stream


def _wrap_idx(idx_stream):
    """[S] int16 -> [128, S/16] wrapped+replicated layout."""
    S = idx_stream.shape[0]
    base = idx_stream.reshape(S // 16, 16).T  # [16, S/16]
    return np.ascontiguousarray(np.tile(base, (8, 1)))


def _per_chunk(stream):
    """[S] -> [128, NCH]: position i=(ch*128+p) -> [p, ch]."""
    NCH = stream.shape[0] // 128
    return np.ascontiguousarray(stream.reshape(NCH, 128).T)


def _prep(inputs):
    edge_index = np.asarray(inputs["edge_index"])
    words_tensor = np.asarray(inputs["words_tensor"])
    user_nodes = np.asarray(inputs["user_nodes"]).astype(np.int64)
    item_nodes = np.asarray(inputs["item_nodes"]).astype(np.int64)

    src = edge_index[0].astype(np.int64)
    dst = edge_index[1].astype(np.int64)
    witem = words_tensor[0].astype(np.int64)
    wword = words_tensor[1].astype(np.int64)

    # sorted-by-dst edge list for per-slot range lookup
    eorder = np.argsort(dst, kind="stable")
    sdst = dst[eorder]
    ssrc = src[eorder]
    worder = np.argsort(witem, kind="stable")
    switem_srt = witem[worder]
    swword = wword[worder]

    # per-core instance lists
    e_data = []
    w_data = []
    cnt_e = np.zeros((NC, NREG_E, NT), np.int64)
    cnt_w = np.zeros((NC, NREG_W, IT), np.int64)
    for c in range(NC):
        slots = np.concatenate([user_nodes[c * BPC:(c + 1) * BPC],
                                item_nodes[c * BPC:(c + 1) * BPC]])  # [2048]
        st = np.searchsorted(sdst, slots)
        en = np.searchsorted(sdst, slots, side="right")
        lens = en - st
        slot_rep = np.repeat(np.arange(NSLOT), lens)
        src_rep = ssrc[_ragged_gather(st, lens)]
        reg_rep = src_rep // REG_E
        loc_rep = src_rep - reg_rep * REG_E
        np.add.at(cnt_e[c], (reg_rep, slot_rep >> 7), 1)
        e_data.append((slot_rep, loc_rep, reg_rep))

        items = item_nodes[c * BPC:(c + 1) * BPC] - NUM_USER  # [1024]
        wst = np.searchsorted(switem_srt, items)
        wen = np.searchsorted(switem_srt, items, side="right")
        wlens = wen - wst
        wslot_rep = np.repeat(np.arange(BPC), wlens)
        word_rep = swword[_ragged_gather(wst, wlens)]
        wreg_rep = word_rep // REG_W
        wloc_rep = word_rep - wreg_rep * REG_W
        np.add.at(cnt_w[c], (wreg_rep, wslot_rep >> 7), 1)
        winv = 1.0 / np.maximum(wlens, 1).astype(np.float64)
        wscale_rep = winv[wslot_rep].astype(np.float32)
        w_data.append((wslot_rep, wloc_rep, wreg_rep, wscale_rep))

    es = _Sched(cnt_e, NT, NREG_E, EB)
    ws = _Sched(cnt_w, IT, NREG_W, WB)

    eidx = np.zeros((NC, 128, es.S // 16), np.int16)
    eloc = np.zeros((NC, 128, es.NCH), bf16)
    widx = np.zeros((NC, 128, ws.S // 16), np.int16)
    wloc = np.zeros((NC, 128, ws.NCH), bf16)
    wsc = np.zeros((NC, 128, ws.NCH), bf16)
    vfT = np.zeros((NC, WDIM, BPC), bf16)
    v_feat = np.asarray(inputs["v_feat"], np.float32)
    for c in range(NC):
        slot_rep, loc_rep, reg_rep = e_data[c]
        i_s, l_s, _ = _fill_stream(es, slot_rep, loc_rep, reg_rep)
        eidx[c] = _wrap_idx(i_s)
        eloc[c] = _per_chunk(l_s)
        wslot_rep, wloc_rep, wreg_rep, wscale_rep = w_data[c]
        i_s, l_s, s_s = _fill_stream(ws, wslot_rep, wloc_rep, wreg_rep,
                                     scale=wscale_rep)
        widx[c] = _wrap_idx(i_s)
        wloc[c] = _per_chunk(l_s)
        wsc[c] = _per_chunk(s_s)
        items = item_nodes[c * BPC:(c + 1) * BPC] - NUM_USER
        vfT[c] = v_feat[items].T.astype(bf16)

    return dict(es=es, ws=ws, eidx=eidx, eloc=eloc,
                widx=widx, wloc=wloc, wsc=wsc, vfT=vfT)


# ------------------------------------------------------------- bass program

def _build_program(es, ws):
    from concourse import bass, bacc, mybir
    import concourse.tile as tile
    dt = mybir.dt

    nc = bacc.Bacc(None, target_bir_lowering=False)
    f32 = dt.float32
    bf = dt.bfloat16

    id_in = nc.dram_tensor("id_emb", [NNODE, DIM], f32, kind="ExternalInput")
    wt_in = nc.dram_tensor("wt_bf", [VOCAB, WDIM], bf, kind="ExternalInput")
    eidx_in = nc.dram_tensor("eidx", [128, es.S // 16], dt.int16, kind="ExternalInput")
    eloc_in = nc.dram_tensor("eloc", [128, es.NCH], bf, kind="ExternalInput")
    widx_in = nc.dram_tensor("widx", [128, ws.S // 16], dt.int16, kind="ExternalInput")
    wloc_in = nc.dram_tensor("wloc", [128, ws.NCH], bf, kind="ExternalInput")
    wsc_in = nc.dram_tensor("wsc", [128, ws.NCH], bf, kind="ExternalInput")
    vfT_in = nc.dram_tensor("vfT", [WDIM, BPC], bf, kind="ExternalInput")
    cw_in = nc.dram_tensor("cw_bf", [DIM, DIM], bf, kind="ExternalInput")
    ww_in = nc.dram_tensor("ww_bf", [DIM, DIM], bf, kind="ExternalInput")
    w2_in = nc.dram_tensor("w2_bf", [DIM, DIM], bf, kind="ExternalInput")
    lw_in = nc.dram_tensor("lw_bf", [2 * WDIM, DIM], bf, kind="ExternalInput")
    lb_in = nc.dram_tensor("lb_col", [DIM, 1], f32, kind="ExternalInput")
    ones_in = nc.dram_tensor("ones_col", [DIM, 1], f32, kind="ExternalInput")
    iota_in = nc.dram_tensor("iota_bf", [128, 128], bf, kind="ExternalInput")
    out = nc.dram_tensor("scores_w", [128, 8], f32, kind="ExternalOutput")

    id_regions = [(0, 23552), (23552, 47104), (47104, 70000)]
    wt_regions = [(r * REG_W, (r + 1) * REG_W) for r in range(NREG_W)]

    with tile.TileContext(nc) as tc:
        with tc.tile_pool(name="const", bufs=1) as cpool, \
             tc.tile_pool(name="persist", bufs=1) as pp, \
             tc.tile_pool(name="ewp", bufs=2) as ewp, \
             tc.tile_pool(name="wwp", bufs=2) as wwp, \
             tc.tile_pool(name="xp", bufs=2) as xp, \
             tc.tile_pool(name="psw", bufs=2, space="PSUM") as psw, \
             tc.tile_pool(name="pse", bufs=3, space="PSUM") as pse, \
             tc.tile_pool(name="psm", bufs=2, space="PSUM") as psm:

            iota = cpool.tile([128, 128], bf)
            cw = cpool.tile([DIM, DIM], bf)
            ww = cpool.tile([DIM, DIM], bf)
            w2 = cpool.tile([DIM, DIM], bf)
            lw = cpool.tile([128, 2 * DIM], bf)   # cols 0:64 = v-half, 64:128 = t-half
            lb = cpool.tile([DIM, 1], f32)
            ones = cpool.tile([DIM, 1], f32)
            nc.sync.dma_start(out=iota[:], in_=iota_in[:])
            nc.sync.dma_start(out=cw[:], in_=cw_in[:])
            nc.sync.dma_start(out=ww[:], in_=ww_in[:])
            nc.sync.dma_start(out=w2[:], in_=w2_in[:])
            nc.sync.dma_start(out=lw[:, 0:DIM], in_=lw_in[0:128, :])
            nc.sync.dma_start(out=lw[:, DIM:2 * DIM], in_=lw_in[128:256, :])
            nc.sync.dma_start(out=lb[:], in_=lb_in[:])
            nc.sync.dma_start(out=ones[:], in_=ones_in[:])

            eidx_sb = pp.tile([128, es.S // 16], dt.int16)
            eloc_sb = pp.tile([128, es.NCH], bf)
            widx_sb = pp.tile([128, ws.S // 16], dt.int16)
            wloc_sb = pp.tile([128, ws.NCH], bf)
            wsc_sb = pp.tile([128, ws.NCH], bf)
            vfT_sb = pp.tile([WDIM, BPC], bf)
            nc.sync.dma_start(out=eidx_sb[:], in_=eidx_in[:])
            nc.sync.dma_start(out=eloc_sb[:], in_=eloc_in[:])
            nc.sync.dma_start(out=widx_sb[:], in_=widx_in[:])
            nc.sync.dma_start(out=wloc_sb[:], in_=wloc_in[:])
            nc.sync.dma_start(out=wsc_sb[:], in_=wsc_in[:])
            nc.sync.dma_start(out=vfT_sb[:], in_=vfT_in[:])

            tfT_sb = pp.tile([WDIM, IT * 128], bf)
            fT_sb = pp.tile([DIM, IT * 128], bf)
            x2T_sb = pp.tile([DIM, NT * 128], bf)
            tfsum_sb = pp.tile([WDIM, IT * 128], f32)
            agg_sb = pp.tile([DIM, NT * 128], f32)
            nc.vector.memset(tfsum_sb[:], 0.0)
            nc.vector.memset(agg_sb[:], 0.0)

            # ---- words: t_feat^T accumulation ----
            wps = None
            for (r, ch0, nb) in ws.batches:
                r0, r1 = wt_regions[r]
                wpay = wwp.tile([128, WB * WDIM], bf, tag="wpay")
                pay3 = wpay[:].rearrange("p (k d) -> p k d", d=WDIM)
                nc.gpsimd.dma_gather(
                    wpay[:, 0:nb * WDIM].rearrange("p (k d) -> p k d", d=WDIM),
                    wt_in[r0:r1, :],
                    widx_sb[:, ch0 * 8:(ch0 + nb) * 8],
                    nb * 128, nb * 128, WDIM, single_packet=False)
                wpays = wwp.tile([128, WB * WDIM], bf, tag="wpays")
                pays3 = wpays[:].rearrange("p (k d) -> p k d", d=WDIM)
                nc.vector.tensor_tensor(
                    out=pays3[:, 0:nb, :], in0=pay3[:, 0:nb, :],
                    in1=wsc_sb[:, ch0:ch0 + nb][:, :, None].to_broadcast(
                        [128, nb, WDIM]),
                    op=mybir.AluOpType.mult)
                woh = wwp.tile([128, WB * 128], bf, tag="woh")
                oh3 = woh[:].rearrange("p (k d) -> p k d", d=128)
                nc.vector.tensor_tensor(
                    out=oh3[:, 0:nb, :],
                    in0=wloc_sb[:, ch0:ch0 + nb][:, :, None].to_broadcast(
                        [128, nb, 128]),
                    in1=iota[:][:, None, :].to_broadcast([128, nb, 128]),
                    op=mybir.AluOpType.is_equal)
                for k in range(nb):
                    ch = ch0 + k
                    t = int(ws.tile_of[ch])
                    if ws.is_first[ch]:
                        wps = psw.tile([WDIM, 128], f32, tag="wp")
                    nc.tensor.matmul(
                        out=wps[:], lhsT=pays3[:, k, :], rhs=oh3[:, k, :],
                        start=ws.is_first[ch], stop=ws.is_last[ch])
                    if ws.is_last[ch]:
                        sl = tfsum_sb[:, t * 128:(t + 1) * 128]
                        nc.vector.tensor_tensor(out=sl, in0=sl, in1=wps[:],
                                                op=mybir.AluOpType.add)

            for t in range(IT):
                nc.scalar.activation(
                    tfT_sb[:, t * 128:(t + 1) * 128],
                    tfsum_sb[:, t * 128:(t + 1) * 128],
                    mybir.ActivationFunctionType.Copy)

            # ---- f^T = lrelu(lw^T cat^T + lb); fh feeds item-tile x2 ----
            for t in range(IT):
                fp = psm.tile([DIM, 128], f32, tag="mm")
                nc.tensor.matmul(out=fp[:], lhsT=lw[:, 0:DIM],
                                 rhs=vfT_sb[:, t * 128:(t + 1) * 128],
                                 start=True, stop=False)
                nc.tensor.matmul(out=fp[:], lhsT=lw[:, DIM:2 * DIM],
                                 rhs=tfT_sb[:, t * 128:(t + 1) * 128],
                                 start=False, stop=True)
                nc.scalar.activation(
                    fT_sb[:, t * 128:(t + 1) * 128], fp[:],
                    mybir.ActivationFunctionType.Lrelu,
                    bias=lb[:], alpha=SLOPE)

            # ---- edges: agg^T accumulation with on-the-fly normalize ----
            if True:
              eps = None
              for (r, ch0, nb) in es.batches:
                r0, r1 = id_regions[r]
                epay = ewp.tile([128, EB * DIM], f32, tag="epay")
                pay3 = epay[:].rearrange("p (k d) -> p k d", d=DIM)
                nc.gpsimd.dma_gather(
                    epay[:, 0:nb * DIM].rearrange("p (k d) -> p k d", d=DIM),
                    id_in[r0:r1, :],
                    eidx_sb[:, ch0 * 8:(ch0 + nb) * 8],
                    nb * 128, nb * 128, DIM, single_packet=False)
                esq = ewp.tile([128, EB * DIM], f32, tag="esq")
                sq3 = esq[:].rearrange("p (k d) -> p k d", d=DIM)
                nc.vector.tensor_tensor(out=sq3[:, 0:nb, :], in0=pay3[:, 0:nb, :],
                                        in1=pay3[:, 0:nb, :],
                                        op=mybir.AluOpType.mult)
                ss = ewp.tile([128, EB], f32, tag="ess")
                nc.vector.reduce_sum(out=ss[:, 0:nb], in_=sq3[:, 0:nb, :],
                                     axis=mybir.AxisListType.X)
                nc.scalar.sqrt(ss[:, 0:nb], ss[:, 0:nb])
                nc.vector.tensor_scalar_max(out=ss[:, 0:nb], in0=ss[:, 0:nb],
                                            scalar1=1e-12)
                nc.vector.reciprocal(ss[:, 0:nb], ss[:, 0:nb])
                epayb = ewp.tile([128, EB * DIM], bf, tag="epayb")
                payb3 = epayb[:].rearrange("p (k d) -> p k d", d=DIM)
                nc.vector.tensor_tensor(
                    out=payb3[:, 0:nb, :], in0=pay3[:, 0:nb, :],
                    in1=ss[:, 0:nb][:, :, None].to_broadcast([128, nb, DIM]),
                    op=mybir.AluOpType.mult)
                eoh = ewp.tile([128, EB * 128], bf, tag="eoh")
                oh3 = eoh[:].rearrange("p (k d) -> p k d", d=128)
                nc.vector.tensor_tensor(
                    out=oh3[:, 0:nb, :],
                    in0=eloc_sb[:, ch0:ch0 + nb][:, :, None].to_broadcast(
                        [128, nb, 128]),
                    in1=iota[:][:, None, :].to_broadcast([128, nb, 128]),
                    op=mybir.AluOpType.is_equal)
                for k in range(nb):
                    ch = ch0 + k
                    t = int(es.tile_of[ch])
                    if es.is_first[ch]:
                        eps = pse.tile([DIM, 128], f32, tag="ep")
                    nc.tensor.matmul(
                        out=eps[:], lhsT=payb3[:, k, :], rhs=oh3[:, k, :],
                        start=es.is_first[ch], stop=es.is_last[ch])
                    if es.is_last[ch]:
                        sl = agg_sb[:, t * 128:(t + 1) * 128]
                        nc.vector.tensor_tensor(out=sl, in0=sl, in1=eps[:],
                                                op=mybir.AluOpType.add)

              # ---- node tail: x2^T = lrelu(ww^T x1^T (+ w2^T f^T)) ----
              for t in range(NT):
                aggT = xp.tile([DIM, 128], bf, tag="aggT")
                nc.scalar.activation(aggT[:], agg_sb[:, t * 128:(t + 1) * 128],
                                     mybir.ActivationFunctionType.Copy)
                x1p = psm.tile([DIM, 128], f32, tag="mm")
                nc.tensor.matmul(out=x1p[:], lhsT=cw[:], rhs=aggT[:],
                                 start=True, stop=True)
                x1T = xp.tile([DIM, 128], bf, tag="x1T")
                nc.scalar.activation(x1T[:], x1p[:],
                                     mybir.ActivationFunctionType.Lrelu,
                                     alpha=SLOPE)
                x2p = psm.tile([DIM, 128], f32, tag="mm")
                nc.tensor.matmul(out=x2p[:], lhsT=ww[:], rhs=x1T[:],
                                 start=True, stop=(t < IT))
                if t >= IT:
                    ti = t - IT
                    nc.tensor.matmul(out=x2p[:], lhsT=w2[:],
                                     rhs=fT_sb[:, ti * 128:(ti + 1) * 128],
                                     start=False, stop=True)
                nc.scalar.activation(x2T_sb[:, t * 128:(t + 1) * 128], x2p[:],
                                     mybir.ActivationFunctionType.Lrelu,
                                     alpha=SLOPE)

            # ---- scores: dot along dim via ones-matmul ----
            with tc.tile_pool(name="pssc", bufs=1, space="PSUM") as pssc:
                x2T3 = x2T_sb[:].rearrange("p (t s) -> p t s", s=128)
                prod = pp.tile([DIM, IT * 128], f32)
                pr3 = prod[:].rearrange("p (t s) -> p t s", s=128)
                nc.vector.tensor_tensor(out=pr3, in0=x2T3[:, 0:IT, :],
                                        in1=x2T3[:, IT:NT, :],
                                        op=mybir.AluOpType.mult)
                scp = pssc.tile([128, 8], f32)
                for j in range(IT):
                    nc.tensor.matmul(out=scp[:, j:j + 1],
                                     lhsT=pr3[:, j, :], rhs=ones[:],
                                     start=(j == 0), stop=(j == IT - 1))
                sc = pp.tile([128, 8], f32)
                nc.scalar.activation(sc[:], scp[:],
                                     mybir.ActivationFunctionType.Copy)
                nc.sync.dma_start(out=out[:], in_=sc[:])

    nc.finalize()
    return nc


# ------------------------------------------------------------------- kernel

def kernel(**inputs):
    from concourse.bass_utils import run_bass_kernel_spmd

    pr = _prep(inputs)
    es, ws = pr["es"], pr["ws"]
    key = es.key() + ws.key()
    if key not in _CACHE:
        _CACHE[key] = _build_program(es, ws)
    nc = _CACHE[key]

    iota_bf = np.broadcast_to(np.arange(128, dtype=bf16), (128, 128)).copy()
    wt_bf = np.asarray(inputs["word_table"], np.float32).astype(bf16)
    lb_col = np.asarray(inputs["lin_b"], np.float32).reshape(DIM, 1).copy()
    ones_col = np.ones((DIM, 1), np.float32)
    cw_bf = np.asarray(inputs["conv_weight"], np.float32).astype(bf16)
    ww_bf = np.asarray(inputs["weight_W"], np.float32).astype(bf16)
    w2_bf = np.asarray(inputs["weight_2"], np.float32).astype(bf16)
    lw_bf = np.asarray(inputs["lin_w"], np.float32).astype(bf16)
    id_emb = np.ascontiguousarray(np.asarray(inputs["id_embedding"], np.float32))

    in_maps = []
    for c in range(NC):
        in_maps.append({
            "id_emb": id_emb,
            "wt_bf": wt_bf,
            "eidx": pr["eidx"][c],
            "eloc": pr["eloc"][c],
            "widx": pr["widx"][c],
            "wloc": pr["wloc"][c],
            "wsc": pr["wsc"][c],
            "vfT": pr["vfT"][c],
            "cw_bf": cw_bf,
            "ww_bf": ww_bf,
            "w2_bf": w2_bf,
            "lw_bf": lw_bf,
            "lb_col": lb_col,
            "ones_col": ones_col,
            "iota_bf": iota_bf,
        })
    res = run_bass_kernel_spmd(nc, in_maps, list(range(NC)))
    scores = np.empty(B, np.float32)
    for c in range(NC):
        w = res.results[c]["scores_w"]           # [128, 8]
        scores[c * BPC:(c + 1) * BPC] = np.asarray(w, np.float32).T.ravel()
    return scores


kernel.run_traced = None  # set by test harness if needed


# revision 18
# speedup vs baseline: 1.6426x; 1.2302x over previous
"""GCMC (gnn_message_passing) Trainium2 Bass kernel, 8-core SPMD, v2.

Strategy (hardcoded for the nn_GCMC_40870908789353 shapes):
- Score-pair sharding: core c owns pairs [1024c, 1024(c+1)). Its 2048 node
  slots (1024 user + 1024 item, duplicates allowed) are the only rows for
  which agg/x2 are computed, so no collective is needed anywhere.
- Gathers use batched dma_gather (SWDGE, int16 idxs) instead of per-chunk
  indirect DMAs: ~30 instructions/core instead of ~850. Tables are split
  into <=32767-row regions (id_emb 3, word_table 4) to fit int16 indices.
- Transposed dataflow: segment-sum one-hot matmuls run as lhsT=payload,
  rhs=one-hot so PSUM holds agg^T / t_feat^T ([dim, slot]); every later
  matmul chains without a single PE transpose. lin_b rides the ACT bias
  port, x1@W and f@w2 accumulate into the same PSUM tile.
- Edge payload rows are L2-normalized on the fly (square/reduce/rsqrt on
  DVE) which folds F.normalize into the gather and kills the full-table
  normalize pass; the scale-mult also casts the payload to bf16.
- Word payload rows are pre-scaled by 1/deg(item slot) (host metadata), so
  the PSUM directly accumulates the mean.
"""
import sys
for p in ("/opt/trn_rl_repo", "/root/.axon_site/_ro/trn_rl_repo"):
    if p not in sys.path:
        sys.path.insert(0, p)
import numpy as np
import ml_dtypes

NC = 8
NUM_USER = 50000
NUM_ITEM = 20000
NNODE = 70000
VOCAB = 100000
DIM = 64
WDIM = 128
B = 8192
BPC = 1024          # pairs per core
NSLOT = 2048        # node slots per core (1024 user + 1024 item)
NT = 16             # node slot tiles (128 slots, for the x-tail)
IT = 8              # item slot tiles (128 slots, for the f-pipeline)
ET = 32             # edge dst groups (64 slots each)
WT = 16             # word dst groups (64 slots each)
SLOTW = 64          # one-hot width per dst group
REG_E = 23552       # id_emb region rows (3 regions)
NREG_E = 3
REG_W = 25000       # word_table region rows (4 regions)
NREG_W = 4
EB = 48             # edge chunks per dma_gather batch
WB = 32             # word chunks per dma_gather batch
SLOPE = 0.01

_CACHE = {}

bf16 = ml_dtypes.bfloat16


# ---------------------------------------------------------------- CPU prep

def _ragged_gather(starts, lens):
    """positions [starts[i], starts[i]+lens[i]) concatenated."""
    tot = int(lens.sum())
    if tot == 0:
        return np.zeros(0, np.int64)
    cum = np.cumsum(lens) - lens
    return np.repeat(starts - cum, lens) + np.arange(tot)


def _build_stream(slot_rep, val_rep, region_rep, n_tiles, n_reg, extra=None):
    """Per-core stream fill given the instance list (slot, table-local idx,
    region). Returns dict with per-(region,tile) counts and a fill closure.
    """
    key = region_rep * n_tiles + (slot_rep >> 7)
    order = np.argsort(key, kind="stable")
    return order, key[order]


class _Sched:
    """Unified SPMD schedule for one gather family."""

    def __init__(self, cnt, n_tiles, n_reg, batch):
        # cnt: [NC, n_reg, n_tiles] instance counts
        nch = np.ceil(cnt / 128.0).astype(np.int64).max(axis=0)  # [n_reg,n_tiles]
        # every tile needs >=1 chunk overall so start/stop exist
        tile_tot = nch.sum(axis=0)
        for t in range(n_tiles):
            if tile_tot[t] == 0:
                nch[0][t] = 1
        self.nch = nch
        self.n_tiles = n_tiles
        self.n_reg = n_reg
        # global chunk order: region-major, tile-minor
        tiles = []
        regions = []
        for r in range(n_reg):
            for t in range(n_tiles):
                tiles += [t] * int(nch[r][t])
                regions += [r] * int(nch[r][t])
        self.tile_of = np.array(tiles, np.int64)
        self.region_of = np.array(regions, np.int64)
        self.NCH = len(tiles)
        self.S = self.NCH * 128
        # start/stop flags per chunk at (region, tile) GROUP granularity: each
        # group is one PSUM accumulation (own bank) closed within its region.
        self.is_first = []
        self.is_last = []
        for ch in range(len(tiles)):
            r, t = regions[ch], tiles[ch]
            self.is_first.append(ch == 0 or (regions[ch - 1], tiles[ch - 1]) != (r, t))
            self.is_last.append(ch == len(tiles) - 1
                                or (regions[ch + 1], tiles[ch + 1]) != (r, t))
        # group (r,t) -> starting chunk
        self.group_ch0 = np.zeros((n_reg, n_tiles), np.int64)
        ch = 0
        for r in range(n_reg):
            for t in range(n_tiles):
                self.group_ch0[r][t] = ch
                ch += int(nch[r][t])
        # batches: split each region's chunk range into <= batch chunks
        self.batches = []  # (region, ch0, nchunks)
        for r in range(n_reg):
            r0 = int(self.group_ch0[r][0])
            r1 = int(self.group_ch0[r + 1][0]) if r + 1 < n_reg else self.NCH
            ch = r0
            while ch < r1:
                nb = min(batch, r1 - ch)
                self.batches.append((r, ch, nb))
                ch += nb

    def key(self):
        return (self.n_tiles, self.n_reg) + tuple(self.nch.ravel().tolist())


def _fill_stream(sched, slot_rep, loc_val, region_rep, scale=None):
    """Place instances into the padded stream. Returns (idx_stream int16,
    loc_stream bf16, scale_stream bf16 or None)."""
    n_tiles = sched.n_tiles
    key = region_rep * n_tiles + (slot_rep >> 6)
    order = np.argsort(key, kind="stable")
    skey = key[order]
    gcnt = np.bincount(skey, minlength=sched.n_reg * n_tiles)
    # position of each sorted instance: group base*128 + within-group offset
    ch0 = sched.group_ch0.ravel()
    base = np.repeat(ch0 * 128, gcnt)
    within = np.arange(len(order)) - np.repeat(np.cumsum(gcnt) - gcnt, gcnt)
    pos = base + within
    idx_stream = np.zeros(sched.S, np.int16)
    idx_stream[pos] = loc_val[order].astype(np.int16)
    loc_stream = np.full(sched.S, -1.0, bf16)
    loc_stream[pos] = (slot_rep[order] & 63).astype(bf16)
    sc_stream = None
    if scale is not None:
        sc_stream = np.zeros(sched.S, bf16)
        sc_stream[pos] = scale[order].astype(bf16)
    return idx_stream, loc_stream, sc_stream


def _wrap_idx(idx_stream):
    """[S] int16 -> [128, S/16] wrapped+replicated layout."""
    S = idx_stream.shape[0]
    base = idx_stream.reshape(S // 16, 16).T  # [16, S/16]
    return np.ascontiguousarray(np.tile(base, (8, 1)))


def _per_chunk(stream):
    """[S] -> [128, NCH]: position i=(ch*128+p) -> [p, ch]."""
    NCH = stream.shape[0] // 128
    return np.ascontiguousarray(stream.reshape(NCH, 128).T)


def _prep(inputs):
    edge_index = np.asarray(inputs["edge_index"])
    words_tensor = np.asarray(inputs["words_tensor"])
    user_nodes = np.asarray(inputs["user_nodes"]).astype(np.int64)
    item_nodes = np.asarray(inputs["item_nodes"]).astype(np.int64)

    src = edge_index[0].astype(np.int64)
    dst = edge_index[1].astype(np.int64)
    witem = words_tensor[0].astype(np.int64)
    wword = words_tensor[1].astype(np.int64)

    # sorted-by-dst edge list for per-slot range lookup
    eorder = np.argsort(dst, kind="stable")
    sdst = dst[eorder]
    ssrc = src[eorder]
    worder = np.argsort(witem, kind="stable")
    switem_srt = witem[worder]
    swword = wword[worder]

    # per-core instance lists
    e_data = []
    w_data = []
    cnt_e = np.zeros((NC, NREG_E, ET), np.int64)
    cnt_w = np.zeros((NC, NREG_W, WT), np.int64)
    for c in range(NC):
        slots = np.concatenate([user_nodes[c * BPC:(c + 1) * BPC],
                                item_nodes[c * BPC:(c + 1) * BPC]])  # [2048]
        st = np.searchsorted(sdst, slots)
        en = np.searchsorted(sdst, slots, side="right")
        lens = en - st
        slot_rep = np.repeat(np.arange(NSLOT), lens)
        src_rep = ssrc[_ragged_gather(st, lens)]
        reg_rep = src_rep // REG_E
        loc_rep = src_rep - reg_rep * REG_E
        np.add.at(cnt_e[c], (reg_rep, slot_rep >> 6), 1)
        e_data.append((slot_rep, loc_rep, reg_rep))

        items = item_nodes[c * BPC:(c + 1) * BPC] - NUM_USER  # [1024]
        wst = np.searchsorted(switem_srt, items)
        wen = np.searchsorted(switem_srt, items, side="right")
        wlens = wen - wst
        wslot_rep = np.repeat(np.arange(BPC), wlens)
        word_rep = swword[_ragged_gather(wst, wlens)]
        wreg_rep = word_rep // REG_W
        wloc_rep = word_rep - wreg_rep * REG_W
        np.add.at(cnt_w[c], (wreg_rep, wslot_rep >> 6), 1)
        winv = 1.0 / np.maximum(wlens, 1).astype(np.float64)
        wscale_rep = winv[wslot_rep].astype(np.float32)
        w_data.append((wslot_rep, wloc_rep, wreg_rep, wscale_rep))

    es = _Sched(cnt_e, ET, NREG_E, EB)
    ws = _Sched(cnt_w, WT, NREG_W, WB)

    eidx = np.zeros((NC, 128, es.S // 16), np.int16)
    eloc = np.zeros((NC, 128, es.NCH), bf16)
    widx = np.zeros((NC, 128, ws.S // 16), np.int16)
    wloc = np.zeros((NC, 128, ws.NCH), bf16)
    wsc = np.zeros((NC, 128, ws.NCH), bf16)
    vfT = np.zeros((NC, WDIM, BPC), bf16)
    v_feat = np.asarray(inputs["v_feat"], np.float32)
    for c in range(NC):
        slot_rep, loc_rep, reg_rep = e_data[c]
        i_s, l_s, _ = _fill_stream(es, slot_rep, loc_rep, reg_rep)
        eidx[c] = _wrap_idx(i_s)
        eloc[c] = _per_chunk(l_s)
        wslot_rep, wloc_rep, wreg_rep, wscale_rep = w_data[c]
        i_s, l_s, s_s = _fill_stream(ws, wslot_rep, wloc_rep, wreg_rep,
                                     scale=wscale_rep)
        widx[c] = _wrap_idx(i_s)
        wloc[c] = _per_chunk(l_s)
        wsc[c] = _per_chunk(s_s)
        items = item_nodes[c * BPC:(c + 1) * BPC] - NUM_USER
        vfT[c] = v_feat[items].T.astype(bf16)

    return dict(es=es, ws=ws, eidx=eidx, eloc=eloc,
                widx=widx, wloc=wloc, wsc=wsc, vfT=vfT)


# ------------------------------------------------------------- bass program

def _build_program(es, ws):
    from concourse import bass, bacc, mybir
    import concourse.tile as tile
    dt = mybir.dt

    nc = bacc.Bacc(None, target_bir_lowering=False, num_swdge_queues=4)
    f32 = dt.float32
    bf = dt.bfloat16

    id_in = nc.dram_tensor("id_emb", [NNODE, DIM], f32, kind="ExternalInput")
    wt_in = nc.dram_tensor("wt_bf", [VOCAB, WDIM], bf, kind="ExternalInput")
    eidx_in = nc.dram_tensor("eidx", [128, es.S // 16], dt.int16, kind="ExternalInput")
    eloc_in = nc.dram_tensor("eloc", [128, es.NCH], bf, kind="ExternalInput")
    widx_in = nc.dram_tensor("widx", [128, ws.S // 16], dt.int16, kind="ExternalInput")
    wloc_in = nc.dram_tensor("wloc", [128, ws.NCH], bf, kind="ExternalInput")
    wsc_in = nc.dram_tensor("wsc", [128, ws.NCH], bf, kind="ExternalInput")
    vfT_in = nc.dram_tensor("vfT", [WDIM, BPC], bf, kind="ExternalInput")
    cw_in = nc.dram_tensor("cw_bf", [DIM, DIM], bf, kind="ExternalInput")
    ww_in = nc.dram_tensor("ww_bf", [DIM, DIM], bf, kind="ExternalInput")
    w2_in = nc.dram_tensor("w2_bf", [DIM, DIM], bf, kind="ExternalInput")
    lw_in = nc.dram_tensor("lw_bf", [2 * WDIM, DIM], bf, kind="ExternalInput")
    lb_in = nc.dram_tensor("lb_col", [DIM, 1], f32, kind="ExternalInput")
    ones_in = nc.dram_tensor("ones_col", [DIM, 1], f32, kind="ExternalInput")
    iota_in = nc.dram_tensor("iota_bf", [128, 128], bf, kind="ExternalInput")
    out = nc.dram_tensor("scores_w", [128, 8], f32, kind="ExternalOutput")

    id_regions = [(0, 23552), (23552, 47104), (47104, 70000)]
    wt_regions = [(r * REG_W, (r + 1) * REG_W) for r in range(NREG_W)]

    with tile.TileContext(nc) as tc:
        with tc.tile_pool(name="const", bufs=1) as cpool, \
             tc.tile_pool(name="persist", bufs=1) as pp, \
             tc.tile_pool(name="ewp", bufs=2) as ewp, \
             tc.tile_pool(name="wwp", bufs=2) as wwp, \
             tc.tile_pool(name="xp", bufs=2) as xp, \
             tc.tile_pool(name="psw", bufs=2, space="PSUM") as psw, \
             tc.tile_pool(name="pse", bufs=3, space="PSUM") as pse, \
             tc.tile_pool(name="psm", bufs=2, space="PSUM") as psm:

            iota = cpool.tile([128, 128], bf)
            cw = cpool.tile([DIM, DIM], bf)
            ww = cpool.tile([DIM, DIM], bf)
            w2 = cpool.tile([DIM, DIM], bf)
            lw = cpool.tile([128, 2 * DIM], bf)   # cols 0:64 = v-half, 64:128 = t-half
            lb = cpool.tile([DIM, 1], f32)
            ones = cpool.tile([DIM, 1], f32)
            nc.sync.dma_start(out=iota[:], in_=iota_in[:])
            nc.sync.dma_start(out=cw[:], in_=cw_in[:])
            nc.sync.dma_start(out=ww[:], in_=ww_in[:])
            nc.sync.dma_start(out=w2[:], in_=w2_in[:])
            nc.sync.dma_start(out=lw[:, 0:DIM], in_=lw_in[0:128, :])
            nc.sync.dma_start(out=lw[:, DIM:2 * DIM], in_=lw_in[128:256, :])
            nc.sync.dma_start(out=lb[:], in_=lb_in[:])
            nc.sync.dma_start(out=ones[:], in_=ones_in[:])

            eidx_sb = pp.tile([128, es.S // 16], dt.int16)
            eloc_sb = pp.tile([128, es.NCH], bf)
            widx_sb = pp.tile([128, ws.S // 16], dt.int16)
            wloc_sb = pp.tile([128, ws.NCH], bf)
            wsc_sb = pp.tile([128, ws.NCH], bf)
            vfT_sb = pp.tile([WDIM, BPC], bf)
            nc.sync.dma_start(out=eidx_sb[:], in_=eidx_in[:])
            nc.sync.dma_start(out=eloc_sb[:], in_=eloc_in[:])
            nc.sync.dma_start(out=widx_sb[:], in_=widx_in[:])
            nc.sync.dma_start(out=wloc_sb[:], in_=wloc_in[:])
            nc.sync.dma_start(out=wsc_sb[:], in_=wsc_in[:])
            nc.sync.dma_start(out=vfT_sb[:], in_=vfT_in[:])

            tfT_sb = pp.tile([WDIM, IT * 128], bf)
            fT_sb = pp.tile([DIM, IT * 128], bf)
            x2T_sb = pp.tile([DIM, NT * 128], bf)
            tfsum_sb = pp.tile([WDIM, IT * 128], f32)
            agg_sb = pp.tile([DIM, NT * 128], f32)
            nc.vector.memset(tfsum_sb[:], 0.0)
            nc.vector.memset(agg_sb[:], 0.0)

            # ---- words: t_feat^T accumulation ----
            wps = None
            for wq, (r, ch0, nb) in enumerate(ws.batches):
                r0, r1 = wt_regions[r]
                wpay = wwp.tile([128, WB * WDIM], bf, tag="wpay")
                pay3 = wpay[:].rearrange("p (k d) -> p k d", d=WDIM)
                nc.gpsimd.dma_gather(
                    wpay[:, 0:nb * WDIM].rearrange("p (k d) -> p k d", d=WDIM),
                    wt_in[r0:r1, :],
                    widx_sb[:, ch0 * 8:(ch0 + nb) * 8],
                    nb * 128, nb * 128, WDIM, single_packet=False,
                    queue_num=wq % 4)
                wpays = wwp.tile([128, WB * WDIM], bf, tag="wpays")
                pays3 = wpays[:].rearrange("p (k d) -> p k d", d=WDIM)
                nc.vector.tensor_tensor(
                    out=pays3[:, 0:nb, :], in0=pay3[:, 0:nb, :],
                    in1=wsc_sb[:, ch0:ch0 + nb][:, :, None].to_broadcast(
                        [128, nb, WDIM]),
                    op=mybir.AluOpType.mult)
                woh = wwp.tile([128, WB * SLOTW], bf, tag="woh")
                oh3 = woh[:].rearrange("p (k d) -> p k d", d=SLOTW)
                nc.vector.tensor_tensor(
                    out=oh3[:, 0:nb, :],
                    in0=wloc_sb[:, ch0:ch0 + nb][:, :, None].to_broadcast(
                        [128, nb, SLOTW]),
                    in1=iota[:][:, None, 0:SLOTW].to_broadcast([128, nb, SLOTW]),
                    op=mybir.AluOpType.is_equal)
                for k in range(nb):
                    ch = ch0 + k
                    t = int(ws.tile_of[ch])
                    if ws.is_first[ch]:
                        wps = psw.tile([WDIM, SLOTW], f32, tag="wp")
                    nc.tensor.matmul(
                        out=wps[:], lhsT=pays3[:, k, :], rhs=oh3[:, k, :],
                        start=ws.is_first[ch], stop=ws.is_last[ch])
                    if ws.is_last[ch]:
                        sl = tfsum_sb[:, t * SLOTW:(t + 1) * SLOTW]
                        nc.vector.tensor_tensor(out=sl, in0=sl, in1=wps[:],
                                                op=mybir.AluOpType.add)

            for t in range(IT):
                nc.scalar.activation(
                    tfT_sb[:, t * 128:(t + 1) * 128],
                    tfsum_sb[:, t * 128:(t + 1) * 128],
                    mybir.ActivationFunctionType.Copy)

            # ---- f^T = lrelu(lw^T cat^T + lb); fh feeds item-tile x2 ----
            for t in range(IT):
                fp = psm.tile([DIM, 128], f32, tag="mm")
                nc.tensor.matmul(out=fp[:], lhsT=lw[:, 0:DIM],
                                 rhs=vfT_sb[:, t * 128:(t + 1) * 128],
                                 start=True, stop=False)
                nc.tensor.matmul(out=fp[:], lhsT=lw[:, DIM:2 * DIM],
                                 rhs=tfT_sb[:, t * 128:(t + 1) * 128],
                                 start=False, stop=True)
                nc.scalar.activation(
                    fT_sb[:, t * 128:(t + 1) * 128], fp[:],
                    mybir.ActivationFunctionType.Lrelu,
                    bias=lb[:], alpha=SLOPE)

            # ---- edges: agg^T accumulation with on-the-fly normalize ----
            if True:
              eps = None
              for eq, (r, ch0, nb) in enumerate(es.batches):
                r0, r1 = id_regions[r]
                epay = ewp.tile([128, EB * DIM], f32, tag="epay")
                pay3 = epay[:].rearrange("p (k d) -> p k d", d=DIM)
                nc.gpsimd.dma_gather(
                    epay[:, 0:nb * DIM].rearrange("p (k d) -> p k d", d=DIM),
                    id_in[r0:r1, :],
                    eidx_sb[:, ch0 * 8:(ch0 + nb) * 8],
                    nb * 128, nb * 128, DIM, single_packet=False,
                    queue_num=eq % 4)
                esq = ewp.tile([128, EB * DIM], f32, tag="esq")
                sq3 = esq[:].rearrange("p (k d) -> p k d", d=DIM)
                nc.vector.tensor_tensor(out=sq3[:, 0:nb, :], in0=pay3[:, 0:nb, :],
                                        in1=pay3[:, 0:nb, :],
                                        op=mybir.AluOpType.mult)
                ss = ewp.tile([128, EB], f32, tag="ess")
                nc.vector.reduce_sum(out=ss[:, 0:nb], in_=sq3[:, 0:nb, :],
                                     axis=mybir.AxisListType.X)
                nc.scalar.sqrt(ss[:, 0:nb], ss[:, 0:nb])
                nc.vector.reciprocal(ss[:, 0:nb], ss[:, 0:nb])
                epayb = ewp.tile([128, EB * DIM], bf, tag="epayb")
                payb3 = epayb[:].rearrange("p (k d) -> p k d", d=DIM)
                nc.vector.tensor_tensor(
                    out=payb3[:, 0:nb, :], in0=pay3[:, 0:nb, :],
                    in1=ss[:, 0:nb][:, :, None].to_broadcast([128, nb, DIM]),
                    op=mybir.AluOpType.mult)
                eoh = ewp.tile([128, EB * SLOTW], bf, tag="eoh")
                oh3 = eoh[:].rearrange("p (k d) -> p k d", d=SLOTW)
                nc.vector.tensor_tensor(
                    out=oh3[:, 0:nb, :],
                    in0=eloc_sb[:, ch0:ch0 + nb][:, :, None].to_broadcast(
                        [128, nb, SLOTW]),
                    in1=iota[:][:, None, 0:SLOTW].to_broadcast([128, nb, SLOTW]),
                    op=mybir.AluOpType.is_equal)
                for k in range(nb):
                    ch = ch0 + k
                    t = int(es.tile_of[ch])
                    if es.is_first[ch]:
                        eps = pse.tile([DIM, SLOTW], f32, tag="ep")
                    nc.tensor.matmul(
                        out=eps[:], lhsT=payb3[:, k, :], rhs=oh3[:, k, :],
                        start=es.is_first[ch], stop=es.is_last[ch])
                    if es.is_last[ch]:
                        sl = agg_sb[:, t * SLOTW:(t + 1) * SLOTW]
                        nc.vector.tensor_tensor(out=sl, in0=sl, in1=eps[:],
                                                op=mybir.AluOpType.add)

              # ---- node tail: x2^T = lrelu(ww^T x1^T (+ w2^T f^T)) ----
              for t in range(NT):
                aggT = xp.tile([DIM, 128], bf, tag="aggT")
                nc.scalar.activation(aggT[:], agg_sb[:, t * 128:(t + 1) * 128],
                                     mybir.ActivationFunctionType.Copy)
                x1p = psm.tile([DIM, 128], f32, tag="mm")
                nc.tensor.matmul(out=x1p[:], lhsT=cw[:], rhs=aggT[:],
                                 start=True, stop=True)
                x1T = xp.tile([DIM, 128], bf, tag="x1T")
                nc.scalar.activation(x1T[:], x1p[:],
                                     mybir.ActivationFunctionType.Lrelu,
                                     alpha=SLOPE)
                x2p = psm.tile([DIM, 128], f32, tag="mm")
                nc.tensor.matmul(out=x2p[:], lhsT=ww[:], rhs=x1T[:],
                                 start=True, stop=(t < IT))
                if t >= IT:
                    ti = t - IT
                    nc.tensor.matmul(out=x2p[:], lhsT=w2[:],
                                     rhs=fT_sb[:, ti * 128:(ti + 1) * 128],
                                     start=False, stop=True)
                nc.scalar.activation(x2T_sb[:, t * 128:(t + 1) * 128], x2p[:],
                                     mybir.ActivationFunctionType.Lrelu,
                                     alpha=SLOPE)

            # ---- scores: dot along dim via ones-matmul ----
            with tc.tile_pool(name="pssc", bufs=1, space="PSUM") as pssc:
                x2T3 = x2T_sb[:].rearrange("p (t s) -> p t s", s=128)
                prod = pp.tile([DIM, IT * 128], f32)
                pr3 = prod[:].rearrange("p (t s) -> p t s", s=128)
                nc.vector.tensor_tensor(out=pr3, in0=x2T3[:, 0:IT, :],
                                        in1=x2T3[:, IT:NT, :],
                                        op=mybir.AluOpType.mult)
                scp = pssc.tile([128, 8], f32)
                for j in range(IT):
                    nc.tensor.matmul(out=scp[:, j:j + 1],
                                     lhsT=pr3[:, j, :], rhs=ones[:],
                                     start=(j == 0), stop=(j == IT - 1))
                sc = pp.tile([128, 8], f32)
                nc.scalar.activation(sc[:], scp[:],
                                     mybir.ActivationFunctionType.Copy)
                nc.sync.dma_start(out=out[:], in_=sc[:])

    nc.finalize()
    return nc


# ------------------------------------------------------------------- kernel

def kernel(**inputs):
    from concourse.bass_utils import run_bass_kernel_spmd

    pr = _prep(inputs)
    es, ws = pr["es"], pr["ws"]
    key = es.key() + ws.key()
    if key not in _CACHE:
        _CACHE[key] = _build_program(es, ws)
    nc = _CACHE[key]

    iota_bf = np.broadcast_to(np.arange(128, dtype=bf16), (128, 128)).copy()
    wt_bf = np.asarray(inputs["word_table"], np.float32).astype(bf16)
    lb_col = np.asarray(inputs["lin_b"], np.float32).reshape(DIM, 1).copy()
    ones_col = np.ones((DIM, 1), np.float32)
    cw_bf = np.asarray(inputs["conv_weight"], np.float32).astype(bf16)
    ww_bf = np.asarray(inputs["weight_W"], np.float32).astype(bf16)
    w2_bf = np.asarray(inputs["weight_2"], np.float32).astype(bf16)
    lw_bf = np.asarray(inputs["lin_w"], np.float32).astype(bf16)
    id_emb = np.ascontiguousarray(np.asarray(inputs["id_embedding"], np.float32))

    in_maps = []
    for c in range(NC):
        in_maps.append({
            "id_emb": id_emb,
            "wt_bf": wt_bf,
            "eidx": pr["eidx"][c],
            "eloc": pr["eloc"][c],
            "widx": pr["widx"][c],
            "wloc": pr["wloc"][c],
            "wsc": pr["wsc"][c],
            "vfT": pr["vfT"][c],
            "cw_bf": cw_bf,
            "ww_bf": ww_bf,
            "w2_bf": w2_bf,
            "lw_bf": lw_bf,
            "lb_col": lb_col,
            "ones_col": ones_col,
            "iota_bf": iota_bf,
        })
    res = run_bass_kernel_spmd(nc, in_maps, list(range(NC)))
    scores = np.empty(B, np.float32)
    for c in range(NC):
        w = res.results[c]["scores_w"]           # [128, 8]
        scores[c * BPC:(c + 1) * BPC] = np.asarray(w, np.float32).T.ravel()
    return scores


kernel.run_traced = None  # set by test harness if needed


# revision 19
# speedup vs baseline: 2.8598x; 1.7410x over previous
"""GCMC (gnn_message_passing) Trainium2 Bass kernel, 8-core SPMD, v2.

Strategy (hardcoded for the nn_GCMC_40870908789353 shapes):
- Score-pair sharding: core c owns pairs [1024c, 1024(c+1)). Its 2048 node
  slots (1024 user + 1024 item, duplicates allowed) are the only rows for
  which agg/x2 are computed, so no collective is needed anywhere.
- Gathers use batched dma_gather (SWDGE, int16 idxs) instead of per-chunk
  indirect DMAs: ~30 instructions/core instead of ~850. Tables are split
  into <=32767-row regions (id_emb 3, word_table 4) to fit int16 indices.
- Transposed dataflow: segment-sum one-hot matmuls run as lhsT=payload,
  rhs=one-hot so PSUM holds agg^T / t_feat^T ([dim, slot]); every later
  matmul chains without a single PE transpose. lin_b rides the ACT bias
  port, x1@W and f@w2 accumulate into the same PSUM tile.
- Edge payload rows are L2-normalized on the fly (square/reduce/rsqrt on
  DVE) which folds F.normalize into the gather and kills the full-table
  normalize pass; the scale-mult also casts the payload to bf16.
- Word payload rows are pre-scaled by 1/deg(item slot) (host metadata), so
  the PSUM directly accumulates the mean.
"""
import sys
for p in ("/opt/trn_rl_repo", "/root/.axon_site/_ro/trn_rl_repo"):
    if p not in sys.path:
        sys.path.insert(0, p)
import numpy as np
import ml_dtypes

NC = 8
NUM_USER = 50000
NUM_ITEM = 20000
NNODE = 70000
VOCAB = 100000
DIM = 64
WDIM = 128
B = 8192
BPC = 1024          # pairs per core
NSLOT = 2048        # node slots per core (1024 user + 1024 item)
NT = 16             # node slot tiles (128 slots, for the x-tail)
IT = 8              # item slot tiles (128 slots, for the f-pipeline)
ET = 32             # edge dst groups (64 slots each)
WT = 16             # word dst groups (64 slots each)
SLOTW = 64          # one-hot width per dst group
REG_E = 23552       # id_emb region rows (3 regions)
NREG_E = 3
REG_W = 25000       # word_table region rows (4 regions)
NREG_W = 4
EB = 32             # edge chunks per dma_gather batch
WB = 24             # word chunks per dma_gather batch
SLOPE = 0.01

_CACHE = {}

bf16 = ml_dtypes.bfloat16


# ---------------------------------------------------------------- CPU prep

def _ragged_gather(starts, lens):
    """positions [starts[i], starts[i]+lens[i]) concatenated."""
    tot = int(lens.sum())
    if tot == 0:
        return np.zeros(0, np.int64)
    cum = np.cumsum(lens) - lens
    return np.repeat(starts - cum, lens) + np.arange(tot)


def _build_stream(slot_rep, val_rep, region_rep, n_tiles, n_reg, extra=None):
    """Per-core stream fill given the instance list (slot, table-local idx,
    region). Returns dict with per-(region,tile) counts and a fill closure.
    """
    key = region_rep * n_tiles + (slot_rep >> 7)
    order = np.argsort(key, kind="stable")
    return order, key[order]


class _Sched:
    """Unified SPMD schedule for one gather family."""

    def __init__(self, cnt, n_tiles, n_reg, batch):
        # cnt: [NC, n_reg, n_tiles] instance counts
        nch = np.ceil(cnt / 128.0).astype(np.int64).max(axis=0)  # [n_reg,n_tiles]
        # every tile needs >=1 chunk overall so start/stop exist
        tile_tot = nch.sum(axis=0)
        for t in range(n_tiles):
            if tile_tot[t] == 0:
                nch[0][t] = 1
        self.nch = nch
        self.n_tiles = n_tiles
        self.n_reg = n_reg
        # global chunk order: region-major, tile-minor
        tiles = []
        regions = []
        for r in range(n_reg):
            for t in range(n_tiles):
                tiles += [t] * int(nch[r][t])
                regions += [r] * int(nch[r][t])
        self.tile_of = np.array(tiles, np.int64)
        self.region_of = np.array(regions, np.int64)
        self.NCH = len(tiles)
        self.S = self.NCH * 128
        # start/stop flags per chunk at (region, tile) GROUP granularity: each
        # group is one PSUM accumulation (own bank) closed within its region.
        self.is_first = []
        self.is_last = []
        for ch in range(len(tiles)):
            r, t = regions[ch], tiles[ch]
            self.is_first.append(ch == 0 or (regions[ch - 1], tiles[ch - 1]) != (r, t))
            self.is_last.append(ch == len(tiles) - 1
                                or (regions[ch + 1], tiles[ch + 1]) != (r, t))
        # group (r,t) -> starting chunk
        self.group_ch0 = np.zeros((n_reg, n_tiles), np.int64)
        ch = 0
        for r in range(n_reg):
            for t in range(n_tiles):
                self.group_ch0[r][t] = ch
                ch += int(nch[r][t])
        # batches: split each region's chunk range into <= batch chunks
        self.batches = []  # (region, ch0, nchunks)
        for r in range(n_reg):
            r0 = int(self.group_ch0[r][0])
            r1 = int(self.group_ch0[r + 1][0]) if r + 1 < n_reg else self.NCH
            ch = r0
            while ch < r1:
                nb = min(batch, r1 - ch)
                self.batches.append((r, ch, nb))
                ch += nb

    def key(self):
        return (self.n_tiles, self.n_reg) + tuple(self.nch.ravel().tolist())


def _fill_stream(sched, slot_rep, loc_val, region_rep, scale=None):
    """Place instances into the padded stream. Returns (idx_stream int16,
    loc_stream bf16, scale_stream bf16 or None)."""
    n_tiles = sched.n_tiles
    key = region_rep * n_tiles + (slot_rep >> 6)
    order = np.argsort(key, kind="stable")
    skey = key[order]
    gcnt = np.bincount(skey, minlength=sched.n_reg * n_tiles)
    # position of each sorted instance: group base*128 + within-group offset
    ch0 = sched.group_ch0.ravel()
    base = np.repeat(ch0 * 128, gcnt)
    within = np.arange(len(order)) - np.repeat(np.cumsum(gcnt) - gcnt, gcnt)
    pos = base + within
    idx_stream = np.zeros(sched.S, np.int16)
    idx_stream[pos] = loc_val[order].astype(np.int16)
    loc_stream = np.full(sched.S, -1.0, bf16)
    loc_stream[pos] = (slot_rep[order] & 63).astype(bf16)
    sc_stream = None
    if scale is not None:
        sc_stream = np.zeros(sched.S, bf16)
        sc_stream[pos] = scale[order].astype(bf16)
    return idx_stream, loc_stream, sc_stream


def _wrap_idx(idx_stream):
    """[S] int16 -> [128, S/16] wrapped+replicated layout."""
    S = idx_stream.shape[0]
    base = idx_stream.reshape(S // 16, 16).T  # [16, S/16]
    return np.ascontiguousarray(np.tile(base, (8, 1)))


def _per_chunk(stream):
    """[S] -> [128, NCH]: position i=(ch*128+p) -> [p, ch]."""
    NCH = stream.shape[0] // 128
    return np.ascontiguousarray(stream.reshape(NCH, 128).T)


def _prep(inputs):
    edge_index = np.asarray(inputs["edge_index"])
    words_tensor = np.asarray(inputs["words_tensor"])
    user_nodes = np.asarray(inputs["user_nodes"]).astype(np.int64)
    item_nodes = np.asarray(inputs["item_nodes"]).astype(np.int64)

    src = edge_index[0].astype(np.int64)
    dst = edge_index[1].astype(np.int64)
    witem = words_tensor[0].astype(np.int64)
    wword = words_tensor[1].astype(np.int64)

    # sorted-by-dst edge list for per-slot range lookup
    eorder = np.argsort(dst, kind="stable")
    sdst = dst[eorder]
    ssrc = src[eorder]
    worder = np.argsort(witem, kind="stable")
    switem_srt = witem[worder]
    swword = wword[worder]

    # per-core instance lists
    e_data = []
    w_data = []
    cnt_e = np.zeros((NC, NREG_E, ET), np.int64)
    cnt_w = np.zeros((NC, NREG_W, WT), np.int64)
    for c in range(NC):
        slots = np.concatenate([user_nodes[c * BPC:(c + 1) * BPC],
                                item_nodes[c * BPC:(c + 1) * BPC]])  # [2048]
        st = np.searchsorted(sdst, slots)
        en = np.searchsorted(sdst, slots, side="right")
        lens = en - st
        slot_rep = np.repeat(np.arange(NSLOT), lens)
        src_rep = ssrc[_ragged_gather(st, lens)]
        reg_rep = src_rep // REG_E
        loc_rep = src_rep - reg_rep * REG_E
        np.add.at(cnt_e[c], (reg_rep, slot_rep >> 6), 1)
        e_data.append((slot_rep, loc_rep, reg_rep))

        items = item_nodes[c * BPC:(c + 1) * BPC] - NUM_USER  # [1024]
        wst = np.searchsorted(switem_srt, items)
        wen = np.searchsorted(switem_srt, items, side="right")
        wlens = wen - wst
        wslot_rep = np.repeat(np.arange(BPC), wlens)
        word_rep = swword[_ragged_gather(wst, wlens)]
        wreg_rep = word_rep // REG_W
        wloc_rep = word_rep - wreg_rep * REG_W
        np.add.at(cnt_w[c], (wreg_rep, wslot_rep >> 6), 1)
        winv = 1.0 / np.maximum(wlens, 1).astype(np.float64)
        wscale_rep = winv[wslot_rep].astype(np.float32)
        w_data.append((wslot_rep, wloc_rep, wreg_rep, wscale_rep))

    es = _Sched(cnt_e, ET, NREG_E, EB)
    ws = _Sched(cnt_w, WT, NREG_W, WB)

    eidx = np.zeros((NC, 128, es.S // 16), np.int16)
    eloc = np.zeros((NC, 128, es.NCH), bf16)
    widx = np.zeros((NC, 128, ws.S // 16), np.int16)
    wloc = np.zeros((NC, 128, ws.NCH), bf16)
    wsc = np.zeros((NC, 128, ws.NCH), bf16)
    vfT = np.zeros((NC, WDIM, BPC), bf16)
    v_feat = np.asarray(inputs["v_feat"], np.float32)
    for c in range(NC):
        slot_rep, loc_rep, reg_rep = e_data[c]
        i_s, l_s, _ = _fill_stream(es, slot_rep, loc_rep, reg_rep)
        eidx[c] = _wrap_idx(i_s)
        eloc[c] = _per_chunk(l_s)
        wslot_rep, wloc_rep, wreg_rep, wscale_rep = w_data[c]
        i_s, l_s, s_s = _fill_stream(ws, wslot_rep, wloc_rep, wreg_rep,
                                     scale=wscale_rep)
        widx[c] = _wrap_idx(i_s)
        wloc[c] = _per_chunk(l_s)
        wsc[c] = _per_chunk(s_s)
        items = item_nodes[c * BPC:(c + 1) * BPC] - NUM_USER
        vfT[c] = v_feat[items].T.astype(bf16)

    return dict(es=es, ws=ws, eidx=eidx, eloc=eloc,
                widx=widx, wloc=wloc, wsc=wsc, vfT=vfT)


# ------------------------------------------------------------- bass program

def _build_program(es, ws):
    from concourse import bass, bacc, mybir
    import concourse.tile as tile
    dt = mybir.dt

    nc = bacc.Bacc(None, target_bir_lowering=False, num_swdge_queues=4)
    f32 = dt.float32
    bf = dt.bfloat16

    id_in = nc.dram_tensor("id_emb", [NNODE, DIM], f32, kind="ExternalInput")
    wt_in = nc.dram_tensor("wt_bf", [VOCAB, WDIM], bf, kind="ExternalInput")
    eidx_in = nc.dram_tensor("eidx", [128, es.S // 16], dt.int16, kind="ExternalInput")
    eloc_in = nc.dram_tensor("eloc", [128, es.NCH], bf, kind="ExternalInput")
    widx_in = nc.dram_tensor("widx", [128, ws.S // 16], dt.int16, kind="ExternalInput")
    wloc_in = nc.dram_tensor("wloc", [128, ws.NCH], bf, kind="ExternalInput")
    wsc_in = nc.dram_tensor("wsc", [128, ws.NCH], bf, kind="ExternalInput")
    vfT_in = nc.dram_tensor("vfT", [WDIM, BPC], bf, kind="ExternalInput")
    cw_in = nc.dram_tensor("cw_bf", [DIM, DIM], bf, kind="ExternalInput")
    ww_in = nc.dram_tensor("ww_bf", [DIM, DIM], bf, kind="ExternalInput")
    w2_in = nc.dram_tensor("w2_bf", [DIM, DIM], bf, kind="ExternalInput")
    lw_in = nc.dram_tensor("lw_bf", [2 * WDIM, DIM], bf, kind="ExternalInput")
    lb_in = nc.dram_tensor("lb_col", [DIM, 1], f32, kind="ExternalInput")
    ones_in = nc.dram_tensor("ones_col", [DIM, 1], f32, kind="ExternalInput")
    iota_in = nc.dram_tensor("iota_bf", [128, 128], bf, kind="ExternalInput")
    out = nc.dram_tensor("scores_w", [128, 8], f32, kind="ExternalOutput")

    id_regions = [(0, 23552), (23552, 47104), (47104, 70000)]
    wt_regions = [(r * REG_W, (r + 1) * REG_W) for r in range(NREG_W)]

    with tile.TileContext(nc) as tc:
        with tc.tile_pool(name="const", bufs=1) as cpool, \
             tc.tile_pool(name="persist", bufs=1) as pp, \
             tc.tile_pool(name="ewp", bufs=4) as ewp, \
             tc.tile_pool(name="wwp", bufs=4) as wwp, \
             tc.tile_pool(name="xp", bufs=2) as xp, \
             tc.tile_pool(name="psw", bufs=2, space="PSUM") as psw, \
             tc.tile_pool(name="pse", bufs=3, space="PSUM") as pse, \
             tc.tile_pool(name="psm", bufs=2, space="PSUM") as psm:

            iota = cpool.tile([128, 128], bf)
            cw = cpool.tile([DIM, DIM], bf)
            ww = cpool.tile([DIM, DIM], bf)
            w2 = cpool.tile([DIM, DIM], bf)
            lw = cpool.tile([128, 2 * DIM], bf)   # cols 0:64 = v-half, 64:128 = t-half
            lb = cpool.tile([DIM, 1], f32)
            ones = cpool.tile([DIM, 1], f32)
            nc.sync.dma_start(out=iota[:], in_=iota_in[:])
            nc.sync.dma_start(out=cw[:], in_=cw_in[:])
            nc.sync.dma_start(out=ww[:], in_=ww_in[:])
            nc.sync.dma_start(out=w2[:], in_=w2_in[:])
            nc.sync.dma_start(out=lw[:, 0:DIM], in_=lw_in[0:128, :])
            nc.sync.dma_start(out=lw[:, DIM:2 * DIM], in_=lw_in[128:256, :])
            nc.sync.dma_start(out=lb[:], in_=lb_in[:])
            nc.sync.dma_start(out=ones[:], in_=ones_in[:])

            eidx_sb = pp.tile([128, es.S // 16], dt.int16)
            eloc_sb = pp.tile([128, es.NCH], bf)
            widx_sb = pp.tile([128, ws.S // 16], dt.int16)
            wloc_sb = pp.tile([128, ws.NCH], bf)
            wsc_sb = pp.tile([128, ws.NCH], bf)
            vfT_sb = pp.tile([WDIM, BPC], bf)
            nc.sync.dma_start(out=eidx_sb[:], in_=eidx_in[:])
            nc.sync.dma_start(out=eloc_sb[:], in_=eloc_in[:])
            nc.sync.dma_start(out=widx_sb[:], in_=widx_in[:])
            nc.sync.dma_start(out=wloc_sb[:], in_=wloc_in[:])
            nc.sync.dma_start(out=wsc_sb[:], in_=wsc_in[:])
            nc.sync.dma_start(out=vfT_sb[:], in_=vfT_in[:])

            tfT_sb = pp.tile([WDIM, IT * 128], bf)
            fT_sb = pp.tile([DIM, IT * 128], bf)
            x2T_sb = pp.tile([DIM, NT * 128], bf)
            tfsum_sb = pp.tile([WDIM, IT * 128], f32)
            agg_sb = pp.tile([DIM, NT * 128], f32)
            nc.vector.memset(tfsum_sb[:], 0.0)
            nc.vector.memset(agg_sb[:], 0.0)

            # ---- words: t_feat^T accumulation ----
            wps = None
            for wq, (r, ch0, nb) in enumerate(ws.batches):
                r0, r1 = wt_regions[r]
                wpay = wwp.tile([128, WB * WDIM], bf, tag="wpay")
                pay3 = wpay[:].rearrange("p (k d) -> p k d", d=WDIM)
                nc.gpsimd.dma_gather(
                    wpay[:, 0:nb * WDIM].rearrange("p (k d) -> p k d", d=WDIM),
                    wt_in[r0:r1, :],
                    widx_sb[:, ch0 * 8:(ch0 + nb) * 8],
                    nb * 128, nb * 128, WDIM, single_packet=False,
                    queue_num=wq % 4)
                wpays = wwp.tile([128, WB * WDIM], bf, tag="wpays")
                pays3 = wpays[:].rearrange("p (k d) -> p k d", d=WDIM)
                nc.vector.tensor_tensor(
                    out=pays3[:, 0:nb, :], in0=pay3[:, 0:nb, :],
                    in1=wsc_sb[:, ch0:ch0 + nb][:, :, None].to_broadcast(
                        [128, nb, WDIM]),
                    op=mybir.AluOpType.mult)
                woh = wwp.tile([128, WB * SLOTW], bf, tag="woh")
                oh3 = woh[:].rearrange("p (k d) -> p k d", d=SLOTW)
                nc.vector.tensor_tensor(
                    out=oh3[:, 0:nb, :],
                    in0=wloc_sb[:, ch0:ch0 + nb][:, :, None].to_broadcast(
                        [128, nb, SLOTW]),
                    in1=iota[:][:, None, 0:SLOTW].to_broadcast([128, nb, SLOTW]),
                    op=mybir.AluOpType.is_equal)
                for k in range(nb):
                    ch = ch0 + k
                    t = int(ws.tile_of[ch])
                    if ws.is_first[ch]:
                        wps = psw.tile([WDIM, SLOTW], f32, tag="wp")
                    nc.tensor.matmul(
                        out=wps[:], lhsT=pays3[:, k, :], rhs=oh3[:, k, :],
                        start=ws.is_first[ch], stop=ws.is_last[ch])
                    if ws.is_last[ch]:
                        sl = tfsum_sb[:, t * SLOTW:(t + 1) * SLOTW]
                        nc.vector.tensor_tensor(out=sl, in0=sl, in1=wps[:],
                                                op=mybir.AluOpType.add)

            for t in range(IT):
                nc.scalar.activation(
                    tfT_sb[:, t * 128:(t + 1) * 128],
                    tfsum_sb[:, t * 128:(t + 1) * 128],
                    mybir.ActivationFunctionType.Copy)

            # ---- f^T = lrelu(lw^T cat^T + lb); fh feeds item-tile x2 ----
            for t in range(IT):
                fp = psm.tile([DIM, 128], f32, tag="mm")
                nc.tensor.matmul(out=fp[:], lhsT=lw[:, 0:DIM],
                                 rhs=vfT_sb[:, t * 128:(t + 1) * 128],
                                 start=True, stop=False)
                nc.tensor.matmul(out=fp[:], lhsT=lw[:, DIM:2 * DIM],
                                 rhs=tfT_sb[:, t * 128:(t + 1) * 128],
                                 start=False, stop=True)
                nc.scalar.activation(
                    fT_sb[:, t * 128:(t + 1) * 128], fp[:],
                    mybir.ActivationFunctionType.Lrelu,
                    bias=lb[:], alpha=SLOPE)

            # ---- edges: agg^T accumulation with on-the-fly normalize ----
            es_has_pair = set()
            for (_r, _c0, _nb) in es.batches:
                _k = 0
                while _k < _nb:
                    _ch = _c0 + _k
                    if (_k + 1 < _nb) and not es.is_first[_ch + 1]:
                        es_has_pair.add((int(es.region_of[_ch]),
                                         int(es.tile_of[_ch])))
                        _k += 2
                    else:
                        _k += 1
            if True:
              eps = None
              for eq, (r, ch0, nb) in enumerate(es.batches):
                r0, r1 = id_regions[r]
                epay = ewp.tile([128, EB * DIM], f32, tag="epay")
                pay3 = epay[:].rearrange("p (k d) -> p k d", d=DIM)
                nc.gpsimd.dma_gather(
                    epay[:, 0:nb * DIM].rearrange("p (k d) -> p k d", d=DIM),
                    id_in[r0:r1, :],
                    eidx_sb[:, ch0 * 8:(ch0 + nb) * 8],
                    nb * 128, nb * 128, DIM, single_packet=False,
                    queue_num=eq % 4)
                esq = ewp.tile([128, EB * DIM], f32, tag="esq")
                sq3 = esq[:].rearrange("p (k d) -> p k d", d=DIM)
                nc.vector.tensor_tensor(out=sq3[:, 0:nb, :], in0=pay3[:, 0:nb, :],
                                        in1=pay3[:, 0:nb, :],
                                        op=mybir.AluOpType.mult)
                ss = ewp.tile([128, EB], f32, tag="ess")
                nc.vector.reduce_sum(out=ss[:, 0:nb], in_=sq3[:, 0:nb, :],
                                     axis=mybir.AxisListType.X)
                nc.scalar.sqrt(ss[:, 0:nb], ss[:, 0:nb])
                nc.vector.reciprocal(ss[:, 0:nb], ss[:, 0:nb])
                epayb = ewp.tile([128, EB * DIM], bf, tag="epayb")
                payb3 = epayb[:].rearrange("p (k d) -> p k d", d=DIM)
                nc.vector.tensor_tensor(
                    out=payb3[:, 0:nb, :], in0=pay3[:, 0:nb, :],
                    in1=ss[:, 0:nb][:, :, None].to_broadcast([128, nb, DIM]),
                    op=mybir.AluOpType.mult)
                eoh = ewp.tile([128, EB * SLOTW], bf, tag="eoh")
                oh3 = eoh[:].rearrange("p (k d) -> p k d", d=SLOTW)
                nc.vector.tensor_tensor(
                    out=oh3[:, 0:nb, :],
                    in0=eloc_sb[:, ch0:ch0 + nb][:, :, None].to_broadcast(
                        [128, nb, SLOTW]),
                    in1=iota[:][:, None, 0:SLOTW].to_broadcast([128, nb, SLOTW]),
                    op=mybir.AluOpType.is_equal)
                k = 0
                while k < nb:
                    ch = ch0 + k
                    t = int(es.tile_of[ch])
                    if es.is_first[ch]:
                        eps = pse.tile([128, 128], f32, tag="ep")
                    pair = (k + 1 < nb) and not es.is_first[ch + 1]
                    if pair:
                        stop = es.is_last[ch + 1]
                        nc.tensor.matmul(
                            out=eps[:],
                            lhsT=epayb[:, k * DIM:(k + 2) * DIM],
                            rhs=eoh[:, k * SLOTW:(k + 2) * SLOTW],
                            start=es.is_first[ch], stop=stop)
                        k += 2
                    else:
                        stop = es.is_last[ch]
                        nc.tensor.matmul(
                            out=eps[0:DIM, 0:SLOTW],
                            lhsT=epayb[:, k * DIM:(k + 1) * DIM],
                            rhs=eoh[:, k * SLOTW:(k + 1) * SLOTW],
                            start=es.is_first[ch], stop=stop)
                        k += 1
                    if stop:
                        g = (int(es.region_of[ch]), t)
                        sl = agg_sb[:, t * SLOTW:(t + 1) * SLOTW]
                        nc.vector.tensor_tensor(out=sl, in0=sl,
                                                in1=eps[0:DIM, 0:SLOTW],
                                                op=mybir.AluOpType.add)
                        if g in es_has_pair:
                            nc.vector.tensor_tensor(
                                out=sl, in0=sl,
                                in1=eps[DIM:128, SLOTW:128],
                                op=mybir.AluOpType.add)

              # ---- node tail: x2^T = lrelu(ww^T x1^T (+ w2^T f^T)) ----
              for t in range(NT):
                aggT = xp.tile([DIM, 128], bf, tag="aggT")
                nc.scalar.activation(aggT[:], agg_sb[:, t * 128:(t + 1) * 128],
                                     mybir.ActivationFunctionType.Copy)
                x1p = psm.tile([DIM, 128], f32, tag="mm")
                nc.tensor.matmul(out=x1p[:], lhsT=cw[:], rhs=aggT[:],
                                 start=True, stop=True)
                x1T = xp.tile([DIM, 128], bf, tag="x1T")
                nc.scalar.activation(x1T[:], x1p[:],
                                     mybir.ActivationFunctionType.Lrelu,
                                     alpha=SLOPE)
                x2p = psm.tile([DIM, 128], f32, tag="mm")
                nc.tensor.matmul(out=x2p[:], lhsT=ww[:], rhs=x1T[:],
                                 start=True, stop=(t < IT))
                if t >= IT:
                    ti = t - IT
                    nc.tensor.matmul(out=x2p[:], lhsT=w2[:],
                                     rhs=fT_sb[:, ti * 128:(ti + 1) * 128],
                                     start=False, stop=True)
                nc.scalar.activation(x2T_sb[:, t * 128:(t + 1) * 128], x2p[:],
                                     mybir.ActivationFunctionType.Lrelu,
                                     alpha=SLOPE)

            # ---- scores: dot along dim via ones-matmul ----
            with tc.tile_pool(name="pssc", bufs=1, space="PSUM") as pssc:
                x2T3 = x2T_sb[:].rearrange("p (t s) -> p t s", s=128)
                prod = pp.tile([DIM, IT * 128], f32)
                pr3 = prod[:].rearrange("p (t s) -> p t s", s=128)
                nc.vector.tensor_tensor(out=pr3, in0=x2T3[:, 0:IT, :],
                                        in1=x2T3[:, IT:NT, :],
                                        op=mybir.AluOpType.mult)
                scp = pssc.tile([128, 8], f32)
                for j in range(IT):
                    nc.tensor.matmul(out=scp[:, j:j + 1],
                                     lhsT=pr3[:, j, :], rhs=ones[:],
                                     start=(j == 0), stop=(j == IT - 1))
                sc = pp.tile([128, 8], f32)
                nc.scalar.activation(sc[:], scp[:],
                                     mybir.ActivationFunctionType.Copy)
                nc.sync.dma_start(out=out[:], in_=sc[:])

    nc.finalize()
    return nc


# ------------------------------------------------------------------- kernel

def kernel(**inputs):
    from concourse.bass_utils import run_bass_kernel_spmd

    pr = _prep(inputs)
    es, ws = pr["es"], pr["ws"]
    key = es.key() + ws.key()
    if key not in _CACHE:
        _CACHE[key] = _build_program(es, ws)
    nc = _CACHE[key]

    iota_bf = np.broadcast_to(np.arange(128, dtype=bf16), (128, 128)).copy()
    wt_bf = np.asarray(inputs["word_table"], np.float32).astype(bf16)
    lb_col = np.asarray(inputs["lin_b"], np.float32).reshape(DIM, 1).copy()
    ones_col = np.ones((DIM, 1), np.float32)
    cw_bf = np.asarray(inputs["conv_weight"], np.float32).astype(bf16)
    ww_bf = np.asarray(inputs["weight_W"], np.float32).astype(bf16)
    w2_bf = np.asarray(inputs["weight_2"], np.float32).astype(bf16)
    lw_bf = np.asarray(inputs["lin_w"], np.float32).astype(bf16)
    id_emb = np.ascontiguousarray(np.asarray(inputs["id_embedding"], np.float32))

    in_maps = []
    for c in range(NC):
        in_maps.append({
            "id_emb": id_emb,
            "wt_bf": wt_bf,
            "eidx": pr["eidx"][c],
            "eloc": pr["eloc"][c],
            "widx": pr["widx"][c],
            "wloc": pr["wloc"][c],
            "wsc": pr["wsc"][c],
            "vfT": pr["vfT"][c],
            "cw_bf": cw_bf,
            "ww_bf": ww_bf,
            "w2_bf": w2_bf,
            "lw_bf": lw_bf,
            "lb_col": lb_col,
            "ones_col": ones_col,
            "iota_bf": iota_bf,
        })
    res = run_bass_kernel_spmd(nc, in_maps, list(range(NC)))
    scores = np.empty(B, np.float32)
    for c in range(NC):
        w = res.results[c]["scores_w"]           # [128, 8]
        scores[c * BPC:(c + 1) * BPC] = np.asarray(w, np.float32).T.ravel()
    return scores


kernel.run_traced = None  # set by test harness if needed


# revision 28
# speedup vs baseline: 3.1987x; 1.1185x over previous
"""GCMC (gnn_message_passing) Trainium2 Bass kernel, 8-core SPMD, v2.

Strategy (hardcoded for the nn_GCMC_40870908789353 shapes):
- Score-pair sharding: core c owns pairs [1024c, 1024(c+1)). Its 2048 node
  slots (1024 user + 1024 item, duplicates allowed) are the only rows for
  which agg/x2 are computed, so no collective is needed anywhere.
- Gathers use batched dma_gather (SWDGE, int16 idxs) instead of per-chunk
  indirect DMAs: ~30 instructions/core instead of ~850. Tables are split
  into <=32767-row regions (id_emb 3, word_table 4) to fit int16 indices.
- Transposed dataflow: segment-sum one-hot matmuls run as lhsT=payload,
  rhs=one-hot so PSUM holds agg^T / t_feat^T ([dim, slot]); every later
  matmul chains without a single PE transpose. lin_b rides the ACT bias
  port, x1@W and f@w2 accumulate into the same PSUM tile.
- Edge payload rows are L2-normalized on the fly (square/reduce/rsqrt on
  DVE) which folds F.normalize into the gather and kills the full-table
  normalize pass; the scale-mult also casts the payload to bf16.
- Word payload rows are pre-scaled by 1/deg(item slot) (host metadata), so
  the PSUM directly accumulates the mean.
"""
import sys
for p in ("/opt/trn_rl_repo", "/root/.axon_site/_ro/trn_rl_repo"):
    if p not in sys.path:
        sys.path.insert(0, p)
import numpy as np
import ml_dtypes

NC = 8
NUM_USER = 50000
NUM_ITEM = 20000
NNODE = 70000
VOCAB = 100000
DIM = 64
WDIM = 128
B = 8192
BPC = 1024          # pairs per core
NSLOT = 2048        # node slots per core (1024 user + 1024 item)
NT = 16             # node slot tiles (128 slots, for the x-tail)
IT = 8              # item slot tiles (128 slots, for the f-pipeline)
ET = 32             # edge dst groups (64 slots each)
WT = 16             # word dst groups (64 slots each)
SLOTW = 64          # one-hot width per dst group
REG_E = 23552       # id_emb region rows (3 regions)
NREG_E = 3
REG_W = 25000       # word_table region rows (4 regions)
NREG_W = 4
EB = 32             # edge chunks per dma_gather batch
WB = 24             # word chunks per dma_gather batch
SLOPE = 0.01

_CACHE = {}

bf16 = ml_dtypes.bfloat16


# ---------------------------------------------------------------- CPU prep

def _ragged_gather(starts, lens):
    """positions [starts[i], starts[i]+lens[i]) concatenated."""
    tot = int(lens.sum())
    if tot == 0:
        return np.zeros(0, np.int64)
    cum = np.cumsum(lens) - lens
    return np.repeat(starts - cum, lens) + np.arange(tot)


def _build_stream(slot_rep, val_rep, region_rep, n_tiles, n_reg, extra=None):
    """Per-core stream fill given the instance list (slot, table-local idx,
    region). Returns dict with per-(region,tile) counts and a fill closure.
    """
    key = region_rep * n_tiles + (slot_rep >> 7)
    order = np.argsort(key, kind="stable")
    return order, key[order]


class _Sched:
    """Unified SPMD schedule for one gather family."""

    def __init__(self, cnt, n_tiles, n_reg, batch):
        # cnt: [NC, n_reg, n_tiles] instance counts
        nch = np.ceil(cnt / 128.0).astype(np.int64).max(axis=0)  # [n_reg,n_tiles]
        # every tile needs >=1 chunk overall so start/stop exist
        tile_tot = nch.sum(axis=0)
        for t in range(n_tiles):
            if tile_tot[t] == 0:
                nch[0][t] = 1
        self.nch = nch
        self.n_tiles = n_tiles
        self.n_reg = n_reg
        # global chunk order: region-major, tile-minor
        tiles = []
        regions = []
        for r in range(n_reg):
            for t in range(n_tiles):
                tiles += [t] * int(nch[r][t])
                regions += [r] * int(nch[r][t])
        self.tile_of = np.array(tiles, np.int64)
        self.region_of = np.array(regions, np.int64)
        self.NCH = len(tiles)
        self.S = self.NCH * 128
        # start/stop flags per chunk at (region, tile) GROUP granularity: each
        # group is one PSUM accumulation (own bank) closed within its region.
        self.is_first = []
        self.is_last = []
        for ch in range(len(tiles)):
            r, t = regions[ch], tiles[ch]
            self.is_first.append(ch == 0 or (regions[ch - 1], tiles[ch - 1]) != (r, t))
            self.is_last.append(ch == len(tiles) - 1
                                or (regions[ch + 1], tiles[ch + 1]) != (r, t))
        # group (r,t) -> starting chunk
        self.group_ch0 = np.zeros((n_reg, n_tiles), np.int64)
        ch = 0
        for r in range(n_reg):
            for t in range(n_tiles):
                self.group_ch0[r][t] = ch
                ch += int(nch[r][t])
        # batches: split each region's chunk range into <= batch chunks
        self.batches = []  # (region, ch0, nchunks)
        for r in range(n_reg):
            r0 = int(self.group_ch0[r][0])
            r1 = int(self.group_ch0[r + 1][0]) if r + 1 < n_reg else self.NCH
            ch = r0
            while ch < r1:
                nb = min(batch, r1 - ch)
                # Never cut a batch right after a group's first chunk: a
                # continuing group would then open with a 64-partition single
                # whose start=True clears has_written only for partitions
                # 0-63, leaving the pairs' q11 half to accumulate onto the
                # PSUM slot's stale contents.
                if ch + nb < r1:
                    last = ch + nb - 1
                    if self.is_first[last] and not self.is_last[last]:
                        nb -= 1
                self.batches.append((r, ch, nb))
                ch += nb

    def key(self):
        return (self.n_tiles, self.n_reg) + tuple(self.nch.ravel().tolist())


def _fill_stream(sched, slot_rep, loc_val, region_rep, scale=None):
    """Place instances into the padded stream. Returns (idx_stream int16,
    loc_stream bf16, scale_stream bf16 or None)."""
    n_tiles = sched.n_tiles
    key = region_rep * n_tiles + (slot_rep >> 6)
    order = np.argsort(key, kind="stable")
    skey = key[order]
    gcnt = np.bincount(skey, minlength=sched.n_reg * n_tiles)
    # position of each sorted instance: group base*128 + within-group offset
    ch0 = sched.group_ch0.ravel()
    base = np.repeat(ch0 * 128, gcnt)
    within = np.arange(len(order)) - np.repeat(np.cumsum(gcnt) - gcnt, gcnt)
    pos = base + within
    idx_stream = np.zeros(sched.S, np.int16)
    idx_stream[pos] = loc_val[order].astype(np.int16)
    loc_stream = np.full(sched.S, -1.0, bf16)
    loc_stream[pos] = (slot_rep[order] & 63).astype(bf16)
    sc_stream = None
    if scale is not None:
        sc_stream = np.zeros(sched.S, bf16)
        sc_stream[pos] = scale[order].astype(bf16)
    return idx_stream, loc_stream, sc_stream


def _wrap_idx(idx_stream):
    """[S] int16 -> [128, S/16] wrapped+replicated layout."""
    S = idx_stream.shape[0]
    base = idx_stream.reshape(S // 16, 16).T  # [16, S/16]
    return np.ascontiguousarray(np.tile(base, (8, 1)))


def _per_chunk(stream):
    """[S] -> [128, NCH]: position i=(ch*128+p) -> [p, ch]."""
    NCH = stream.shape[0] // 128
    return np.ascontiguousarray(stream.reshape(NCH, 128).T)


def _prep(inputs):
    edge_index = np.asarray(inputs["edge_index"])
    words_tensor = np.asarray(inputs["words_tensor"])
    user_nodes = np.asarray(inputs["user_nodes"]).astype(np.int64)
    item_nodes = np.asarray(inputs["item_nodes"]).astype(np.int64)

    src = edge_index[0].astype(np.int64)
    dst = edge_index[1].astype(np.int64)
    witem = words_tensor[0].astype(np.int64)
    wword = words_tensor[1].astype(np.int64)

    eorder = np.argsort(dst, kind="stable")
    sdst = dst[eorder]
    ssrc = src[eorder]
    worder = np.argsort(witem, kind="stable")
    switem_srt = witem[worder]
    swword = wword[worder]

    deg = np.bincount(dst, minlength=NNODE)
    wc_item = np.bincount(witem, minlength=NUM_ITEM)

    def snake_pos(n):
        i = np.arange(n)
        rnd, lane = divmod(i, 16)
        g = np.where(rnd % 2 == 0, lane, 15 - lane)
        return g * 64 + rnd

    # cluster pairs by item: each item's aggregation lands on one core
    gorder = np.argsort(item_nodes, kind="stable")

    e_data, w_data = [], []
    outperm = np.zeros((NC, BPC), np.int64)
    sidx = np.zeros((NC, 128, BPC // 16), np.int16)
    cnt_e = np.zeros((NC, NREG_E, ET), np.int64)
    cnt_w = np.zeros((NC, NREG_W, WT), np.int64)
    vfT = np.zeros((NC, WDIM, BPC), bf16)
    v_feat = np.asarray(inputs["v_feat"], np.float32)
    for c in range(NC):
        P = gorder[c * BPC:(c + 1) * BPC]
        users = user_nodes[P]
        items = item_nodes[P]
        # user position permutation (balance by degree, snake)
        order_u = np.argsort(-deg[users], kind="stable")
        pos_u = snake_pos(BPC)
        uperm = np.empty(BPC, np.int64)       # uperm[position] = pair rank in P
        uperm[pos_u] = order_u
        outperm[c] = P[uperm]
        # unique items -> balanced slot positions
        uit = np.unique(items)                # sorted node ids
        nu = len(uit)
        iid = uit - NUM_USER
        order_i = np.argsort(-(deg[uit] + wc_item[iid]), kind="stable")
        # rank r (in uit order) -> its balance order index, then snake position
        inv = np.empty(nu, np.int64)
        inv[order_i] = np.arange(nu)
        ipos_of_rank = snake_pos(nu)[inv]
        # final-score gather: position q -> item slot position
        islot_of_pair = ipos_of_rank[np.searchsorted(uit, items[uperm])]
        st16 = islot_of_pair.astype(np.int16)
        sidx[c] = np.ascontiguousarray(
            np.tile(st16.reshape(BPC // 16, 16).T, (8, 1)))

        # edge instances: user positions + unique-item slots
        nodes_e = np.concatenate([users[uperm], uit])
        slots_e = np.concatenate([np.arange(BPC), BPC + ipos_of_rank])
        st = np.searchsorted(sdst, nodes_e)
        en = np.searchsorted(sdst, nodes_e, side="right")
        lens = en - st
        slot_rep = np.repeat(slots_e, lens)
        src_rep = ssrc[_ragged_gather(st, lens)]
        reg_rep = src_rep // REG_E
        loc_rep = src_rep - reg_rep * REG_E
        np.add.at(cnt_e[c], (reg_rep, slot_rep >> 6), 1)
        e_data.append((slot_rep, loc_rep, reg_rep))

        # word instances per unique item
        wst = np.searchsorted(switem_srt, iid)
        wen = np.searchsorted(switem_srt, iid, side="right")
        wlens = wen - wst
        wslot_rep = np.repeat(ipos_of_rank, wlens)
        word_rep = swword[_ragged_gather(wst, wlens)]
        wreg_rep = word_rep // REG_W
        wloc_rep = word_rep - wreg_rep * REG_W
        np.add.at(cnt_w[c], (wreg_rep, wslot_rep >> 6), 1)
        winv = (1.0 / np.maximum(wlens, 1)).astype(np.float32)
        wscale_rep = np.repeat(winv, wlens)
        w_data.append((wslot_rep, wloc_rep, wreg_rep, wscale_rep))

        vf_pos = np.zeros((BPC, WDIM), np.float32)
        vf_pos[ipos_of_rank] = v_feat[iid]
        vfT[c] = vf_pos.T.astype(bf16)

    es = _Sched(cnt_e, ET, NREG_E, EB)
    ws = _Sched(cnt_w, WT, NREG_W, WB)

    eidx = np.zeros((NC, 128, es.S // 16), np.int16)
    eloc = np.zeros((NC, 128, es.NCH), bf16)
    widx = np.zeros((NC, 128, ws.S // 16), np.int16)
    wloc = np.zeros((NC, 128, ws.NCH), bf16)
    wsc = np.zeros((NC, 128, ws.NCH), bf16)
    for c in range(NC):
        slot_rep, loc_rep, reg_rep = e_data[c]
        i_s, l_s, _ = _fill_stream(es, slot_rep, loc_rep, reg_rep)
        eidx[c] = _wrap_idx(i_s)
        eloc[c] = _per_chunk(l_s)
        wslot_rep, wloc_rep, wreg_rep, wscale_rep = w_data[c]
        i_s, l_s, s_s = _fill_stream(ws, wslot_rep, wloc_rep, wreg_rep,
                                     scale=wscale_rep)
        widx[c] = _wrap_idx(i_s)
        wloc[c] = _per_chunk(l_s)
        wsc[c] = _per_chunk(s_s)

    return dict(es=es, ws=ws, eidx=eidx, eloc=eloc,
                widx=widx, wloc=wloc, wsc=wsc, vfT=vfT,
                sidx=sidx, outperm=outperm)


# ------------------------------------------------------------- bass program

def _build_program(es, ws):
    from concourse import bass, bacc, mybir
    import concourse.tile as tile
    dt = mybir.dt

    nc = bacc.Bacc(None, target_bir_lowering=False, num_swdge_queues=4)
    f32 = dt.float32
    bf = dt.bfloat16

    id_in = nc.dram_tensor("id_emb", [NNODE, DIM], f32, kind="ExternalInput")
    wt_in = nc.dram_tensor("wt_bf", [VOCAB, WDIM], bf, kind="ExternalInput")
    eidx_in = nc.dram_tensor("eidx", [128, es.S // 16], dt.int16, kind="ExternalInput")
    eloc_in = nc.dram_tensor("eloc", [128, es.NCH], bf, kind="ExternalInput")
    widx_in = nc.dram_tensor("widx", [128, ws.S // 16], dt.int16, kind="ExternalInput")
    wloc_in = nc.dram_tensor("wloc", [128, ws.NCH], bf, kind="ExternalInput")
    wsc_in = nc.dram_tensor("wsc", [128, ws.NCH], bf, kind="ExternalInput")
    vfT_in = nc.dram_tensor("vfT", [WDIM, BPC], bf, kind="ExternalInput")
    cw_in = nc.dram_tensor("cw_bf", [DIM, DIM], bf, kind="ExternalInput")
    ww_in = nc.dram_tensor("ww_bf", [DIM, DIM], bf, kind="ExternalInput")
    w2_in = nc.dram_tensor("w2_bf", [DIM, DIM], bf, kind="ExternalInput")
    lw_in = nc.dram_tensor("lw_bf", [2 * WDIM, DIM], bf, kind="ExternalInput")
    lb_in = nc.dram_tensor("lb_col", [DIM, 1], f32, kind="ExternalInput")
    ident_in = nc.dram_tensor("ident", [128, 128], f32, kind="ExternalInput")
    sidx_in = nc.dram_tensor("sidx", [128, BPC // 16], dt.int16, kind="ExternalInput")
    iota_in = nc.dram_tensor("iota_bf", [128, 128], bf, kind="ExternalInput")
    out = nc.dram_tensor("scores_w", [128, 8], f32, kind="ExternalOutput")
    x2i_dram = nc.dram_tensor("x2i", [BPC, DIM], f32)
    import os
    DBG = os.environ.get("KDBG") == "1"
    if DBG:
        dbg_agg = nc.dram_tensor("dbg_agg", [DIM, NT * 128], f32, kind="ExternalOutput")
        dbg_tf = nc.dram_tensor("dbg_tf", [WDIM, IT * 128], f32, kind="ExternalOutput")
        dbg_x2 = nc.dram_tensor("dbg_x2", [DIM, NT * 128], f32, kind="ExternalOutput")
        dbg_ipay = nc.dram_tensor("dbg_ipay", [128, IT * DIM], f32, kind="ExternalOutput")

    id_regions = [(0, 23552), (23552, 47104), (47104, 70000)]
    wt_regions = [(r * REG_W, (r + 1) * REG_W) for r in range(NREG_W)]

    with tile.TileContext(nc) as tc:
        with tc.tile_pool(name="const", bufs=1) as cpool, \
             tc.tile_pool(name="persist", bufs=1) as pp, \
             tc.tile_pool(name="ewp", bufs=4) as ewp, \
             tc.tile_pool(name="wwp", bufs=4) as wwp, \
             tc.tile_pool(name="mid", bufs=2) as midp, \
             tc.tile_pool(name="xp", bufs=2) as xp, \
             tc.tile_pool(name="psw", bufs=2, space="PSUM") as psw, \
             tc.tile_pool(name="pse", bufs=2, space="PSUM") as pse, \
             tc.tile_pool(name="psm", bufs=2, space="PSUM") as psm:

            iota = cpool.tile([128, 128], bf)
            cw = cpool.tile([DIM, DIM], bf)
            ww = cpool.tile([DIM, DIM], bf)
            w2 = cpool.tile([DIM, DIM], bf)
            lw = cpool.tile([128, 2 * DIM], bf)   # cols 0:64 = v-half, 64:128 = t-half
            lb = cpool.tile([DIM, 1], f32)
            ident = cpool.tile([128, 128], f32)
            sidx_sb = cpool.tile([128, BPC // 16], dt.int16)
            nc.sync.dma_start(out=iota[:], in_=iota_in[:])
            nc.sync.dma_start(out=cw[:], in_=cw_in[:])
            nc.sync.dma_start(out=ww[:], in_=ww_in[:])
            nc.sync.dma_start(out=w2[:], in_=w2_in[:])
            nc.sync.dma_start(out=lw[:, 0:DIM], in_=lw_in[0:128, :])
            nc.sync.dma_start(out=lw[:, DIM:2 * DIM], in_=lw_in[128:256, :])
            nc.sync.dma_start(out=lb[:], in_=lb_in[:])
            nc.sync.dma_start(out=ident[:], in_=ident_in[:])
            nc.sync.dma_start(out=sidx_sb[:], in_=sidx_in[:])

            eidx_sb = pp.tile([128, es.S // 16], dt.int16)
            eloc_sb = pp.tile([128, es.NCH], bf)
            widx_sb = pp.tile([128, ws.S // 16], dt.int16)
            wloc_sb = pp.tile([128, ws.NCH], bf)
            wsc_sb = pp.tile([128, ws.NCH], bf)
            vfT_sb = pp.tile([WDIM, BPC], bf)
            nc.sync.dma_start(out=eidx_sb[:], in_=eidx_in[:])
            nc.sync.dma_start(out=eloc_sb[:], in_=eloc_in[:])
            nc.sync.dma_start(out=widx_sb[:], in_=widx_in[:])
            nc.sync.dma_start(out=wloc_sb[:], in_=wloc_in[:])
            nc.sync.dma_start(out=wsc_sb[:], in_=wsc_in[:])
            nc.sync.dma_start(out=vfT_sb[:], in_=vfT_in[:])

            tfT_sb = pp.tile([WDIM, IT * 128], bf)
            fT_sb = pp.tile([DIM, IT * 128], bf)
            x2T_sb = pp.tile([DIM, NT * 128], f32)
            tfsum_sb = pp.tile([WDIM, IT * 128], f32)
            agg_sb = pp.tile([DIM, NT * 128], f32)
            nc.vector.memset(tfsum_sb[:], 0.0)
            nc.vector.memset(agg_sb[:], 0.0)

            # ---- words: t_feat^T accumulation ----
            wps = None
            for wq, (r, ch0, nb) in enumerate(ws.batches):
                r0, r1 = wt_regions[r]
                wpay = wwp.tile([128, WB * WDIM], bf, tag="wpay")
                pay3 = wpay[:].rearrange("p (k d) -> p k d", d=WDIM)
                nc.gpsimd.dma_gather(
                    wpay[:, 0:nb * WDIM].rearrange("p (k d) -> p k d", d=WDIM),
                    wt_in[r0:r1, :],
                    widx_sb[:, ch0 * 8:(ch0 + nb) * 8],
                    nb * 128, nb * 128, WDIM, single_packet=False,
                    queue_num=wq % 4)
                wpays = midp.tile([128, WB * WDIM], bf, tag="wpays")
                pays3 = wpays[:].rearrange("p (k d) -> p k d", d=WDIM)
                nc.vector.tensor_tensor(
                    out=pays3[:, 0:nb, :], in0=pay3[:, 0:nb, :],
                    in1=wsc_sb[:, ch0:ch0 + nb][:, :, None].to_broadcast(
                        [128, nb, WDIM]),
                    op=mybir.AluOpType.mult)
                woh = wwp.tile([128, WB * SLOTW], bf, tag="woh")
                oh3 = woh[:].rearrange("p (k d) -> p k d", d=SLOTW)
                nc.vector.tensor_tensor(
                    out=oh3[:, 0:nb, :],
                    in0=wloc_sb[:, ch0:ch0 + nb][:, :, None].to_broadcast(
                        [128, nb, SLOTW]),
                    in1=iota[:][:, None, 0:SLOTW].to_broadcast([128, nb, SLOTW]),
                    op=mybir.AluOpType.is_equal)
                for k in range(nb):
                    ch = ch0 + k
                    t = int(ws.tile_of[ch])
                    if ws.is_first[ch]:
                        wps = psw.tile([WDIM, 512], f32, tag="wp")
                    nc.tensor.matmul(
                        out=wps[:, 0:SLOTW], lhsT=pays3[:, k, :], rhs=oh3[:, k, :],
                        start=ws.is_first[ch], stop=ws.is_last[ch])
                    if ws.is_last[ch]:
                        sl = tfsum_sb[:, t * SLOTW:(t + 1) * SLOTW]
                        nc.vector.tensor_tensor(out=sl, in0=sl,
                                                in1=wps[:, 0:SLOTW],
                                                op=mybir.AluOpType.add)

            for t in range(IT):
                nc.scalar.activation(
                    tfT_sb[:, t * 128:(t + 1) * 128],
                    tfsum_sb[:, t * 128:(t + 1) * 128],
                    mybir.ActivationFunctionType.Copy)

            # ---- f^T = lrelu(lw^T cat^T + lb); fh feeds item-tile x2 ----
            for t in range(IT):
                fp = psm.tile([DIM, 512], f32, tag="mm")
                nc.tensor.matmul(out=fp[:, 0:128], lhsT=lw[:, 0:DIM],
                                 rhs=vfT_sb[:, t * 128:(t + 1) * 128],
                                 start=True, stop=False)
                nc.tensor.matmul(out=fp[:, 0:128], lhsT=lw[:, DIM:2 * DIM],
                                 rhs=tfT_sb[:, t * 128:(t + 1) * 128],
                                 start=False, stop=True)
                nc.scalar.activation(
                    fT_sb[:, t * 128:(t + 1) * 128], fp[:, 0:128],
                    mybir.ActivationFunctionType.Lrelu,
                    bias=lb[:], alpha=SLOPE)

            # ---- edges: agg^T accumulation with on-the-fly normalize ----
            es_has_pair = set()
            for (_r, _c0, _nb) in es.batches:
                _k = 0
                while _k < _nb:
                    _ch = _c0 + _k
                    if (_k + 1 < _nb) and not es.is_first[_ch + 1]:
                        es_has_pair.add((int(es.region_of[_ch]),
                                         int(es.tile_of[_ch])))
                        _k += 2
                    else:
                        _k += 1
            if True:
              eps = None
              for eq, (r, ch0, nb) in enumerate(es.batches):
                r0, r1 = id_regions[r]
                epay = ewp.tile([128, EB * DIM], f32, tag="epay")
                pay3 = epay[:].rearrange("p (k d) -> p k d", d=DIM)
                nc.gpsimd.dma_gather(
                    epay[:, 0:nb * DIM].rearrange("p (k d) -> p k d", d=DIM),
                    id_in[r0:r1, :],
                    eidx_sb[:, ch0 * 8:(ch0 + nb) * 8],
                    nb * 128, nb * 128, DIM, single_packet=False,
                    queue_num=eq % 4)
                esq = midp.tile([128, EB * DIM], f32, tag="esq")
                sq3 = esq[:].rearrange("p (k d) -> p k d", d=DIM)
                nc.vector.tensor_tensor(out=sq3[:, 0:nb, :], in0=pay3[:, 0:nb, :],
                                        in1=pay3[:, 0:nb, :],
                                        op=mybir.AluOpType.mult)
                ss = ewp.tile([128, EB], f32, tag="ess")
                nc.vector.reduce_sum(out=ss[:, 0:nb], in_=sq3[:, 0:nb, :],
                                     axis=mybir.AxisListType.X)
                nc.scalar.sqrt(ss[:, 0:nb], ss[:, 0:nb])
                nc.vector.reciprocal(ss[:, 0:nb], ss[:, 0:nb])
                epayb = ewp.tile([128, EB * DIM], bf, tag="epayb")
                payb3 = epayb[:].rearrange("p (k d) -> p k d", d=DIM)
                nc.vector.tensor_tensor(
                    out=payb3[:, 0:nb, :], in0=pay3[:, 0:nb, :],
                    in1=ss[:, 0:nb][:, :, None].to_broadcast([128, nb, DIM]),
                    op=mybir.AluOpType.mult)
                eoh = ewp.tile([128, EB * SLOTW], bf, tag="eoh")
                oh3 = eoh[:].rearrange("p (k d) -> p k d", d=SLOTW)
                nc.vector.tensor_tensor(
                    out=oh3[:, 0:nb, :],
                    in0=eloc_sb[:, ch0:ch0 + nb][:, :, None].to_broadcast(
                        [128, nb, SLOTW]),
                    in1=iota[:][:, None, 0:SLOTW].to_broadcast([128, nb, SLOTW]),
                    op=mybir.AluOpType.is_equal)
                k = 0
                while k < nb:
                    ch = ch0 + k
                    t = int(es.tile_of[ch])
                    if es.is_first[ch]:
                        eps = pse.tile([128, 512], f32, tag="ep")
                    pair = (k + 1 < nb) and not es.is_first[ch + 1]
                    if pair:
                        stop = es.is_last[ch + 1]
                        nc.tensor.matmul(
                            out=eps[:, 0:128],
                            lhsT=epayb[:, k * DIM:(k + 2) * DIM],
                            rhs=eoh[:, k * SLOTW:(k + 2) * SLOTW],
                            start=es.is_first[ch], stop=stop)
                        k += 2
                    else:
                        stop = es.is_last[ch]
                        nc.tensor.matmul(
                            out=eps[0:DIM, 0:SLOTW],
                            lhsT=epayb[:, k * DIM:(k + 1) * DIM],
                            rhs=eoh[:, k * SLOTW:(k + 1) * SLOTW],
                            start=es.is_first[ch], stop=stop)
                        k += 1
                    if stop:
                        g = (int(es.region_of[ch]), t)
                        sl = agg_sb[:, t * SLOTW:(t + 1) * SLOTW]
                        nc.vector.tensor_tensor(out=sl, in0=sl,
                                                in1=eps[0:DIM, 0:SLOTW],
                                                op=mybir.AluOpType.add)
                        if g in es_has_pair:
                            nc.vector.tensor_tensor(
                                out=sl, in0=sl,
                                in1=eps[DIM:128, SLOTW:128],
                                op=mybir.AluOpType.add)

              # ---- node tail: x2^T = lrelu(ww^T x1^T (+ w2^T f^T)) ----
              for t in range(NT):
                aggT = xp.tile([DIM, 128], bf, tag="aggT")
                nc.scalar.activation(aggT[:], agg_sb[:, t * 128:(t + 1) * 128],
                                     mybir.ActivationFunctionType.Copy)
                x1p = psm.tile([DIM, 512], f32, tag="mm")
                nc.tensor.matmul(out=x1p[:, 0:128], lhsT=cw[:], rhs=aggT[:],
                                 start=True, stop=True)
                x1T = xp.tile([DIM, 128], bf, tag="x1T")
                nc.scalar.activation(x1T[:], x1p[:, 0:128],
                                     mybir.ActivationFunctionType.Lrelu,
                                     alpha=SLOPE)
                x2p = psm.tile([DIM, 512], f32, tag="mm")
                nc.tensor.matmul(out=x2p[:, 0:128], lhsT=ww[:], rhs=x1T[:],
                                 start=True, stop=(t < IT))
                if t >= IT:
                    ti = t - IT
                    nc.tensor.matmul(out=x2p[:, 0:128], lhsT=w2[:],
                                     rhs=fT_sb[:, ti * 128:(ti + 1) * 128],
                                     start=False, stop=True)
                nc.scalar.activation(x2T_sb[:, t * 128:(t + 1) * 128], x2p[:, 0:128],
                                     mybir.ActivationFunctionType.Lrelu,
                                     alpha=SLOPE)

            # ---- scores: transpose x2^T tiles to rows, route item rows ----
            x2r_u = pp.tile([128, IT * DIM], f32)
            x2r_i = pp.tile([128, IT * DIM], f32)
            for t in range(IT):
                ps_t = psm.tile([128, 512], f32, tag="tr")
                nc.tensor.transpose(out=ps_t[:, 0:DIM],
                                    in_=x2T_sb[:, t * 128:(t + 1) * 128],
                                    identity=ident[0:DIM, 0:DIM])
                nc.scalar.activation(x2r_u[:, t * DIM:(t + 1) * DIM], ps_t[:, 0:DIM],
                                     mybir.ActivationFunctionType.Copy)
            for t in range(IT):
                ps_t = psm.tile([128, 512], f32, tag="tr")
                nc.tensor.transpose(out=ps_t[:, 0:DIM],
                                    in_=x2T_sb[:, (IT + t) * 128:(IT + t + 1) * 128],
                                    identity=ident[0:DIM, 0:DIM])
                nc.scalar.activation(x2r_i[:, t * DIM:(t + 1) * DIM], ps_t[:, 0:DIM],
                                     mybir.ActivationFunctionType.Copy)
            nc.sync.dma_start(
                out=x2i_dram[:, :].rearrange("(t p) d -> p t d", p=128),
                in_=x2r_i[:].rearrange("p (t d) -> p t d", d=DIM))
            ipay = pp.tile([128, IT * DIM], f32)
            nc.gpsimd.dma_gather(
                ipay[:].rearrange("p (k d) -> p k d", d=DIM),
                x2i_dram[:, :],
                sidx_sb[:],
                BPC, BPC, DIM, single_packet=False)
            prod = pp.tile([128, IT * DIM], f32)
            nc.vector.tensor_tensor(
                out=prod[:].rearrange("p (k d) -> p k d", d=DIM),
                in0=x2r_u[:].rearrange("p (k d) -> p k d", d=DIM),
                in1=ipay[:].rearrange("p (k d) -> p k d", d=DIM),
                op=mybir.AluOpType.mult)
            sc = pp.tile([128, 8], f32)
            nc.vector.reduce_sum(out=sc[:],
                                 in_=prod[:].rearrange("p (k d) -> p k d", d=DIM),
                                 axis=mybir.AxisListType.X)
            nc.sync.dma_start(out=out[:], in_=sc[:])
            if DBG:
                nc.sync.dma_start(out=dbg_agg[:], in_=agg_sb[:])
                nc.sync.dma_start(out=dbg_tf[:], in_=tfsum_sb[:])
                nc.sync.dma_start(out=dbg_x2[:], in_=x2T_sb[:])
                nc.sync.dma_start(out=dbg_ipay[:], in_=ipay[:])

    nc.finalize()
    return nc


# ------------------------------------------------------------------- kernel

def kernel(**inputs):
    from concourse.bass_utils import run_bass_kernel_spmd

    pr = _prep(inputs)
    es, ws = pr["es"], pr["ws"]
    key = es.key() + ws.key()
    if key not in _CACHE:
        _CACHE[key] = _build_program(es, ws)
    nc = _CACHE[key]

    iota_bf = np.broadcast_to(np.arange(128, dtype=bf16), (128, 128)).copy()
    ident = np.eye(128, dtype=np.float32)
    wt_bf = np.asarray(inputs["word_table"], np.float32).astype(bf16)
    lb_col = np.asarray(inputs["lin_b"], np.float32).reshape(DIM, 1).copy()
    cw_bf = np.asarray(inputs["conv_weight"], np.float32).astype(bf16)
    ww_bf = np.asarray(inputs["weight_W"], np.float32).astype(bf16)
    w2_bf = np.asarray(inputs["weight_2"], np.float32).astype(bf16)
    lw_bf = np.asarray(inputs["lin_w"], np.float32).astype(bf16)
    id_emb = np.ascontiguousarray(np.asarray(inputs["id_embedding"], np.float32))

    in_maps = []
    for c in range(NC):
        in_maps.append({
            "id_emb": id_emb,
            "wt_bf": wt_bf,
            "eidx": pr["eidx"][c],
            "eloc": pr["eloc"][c],
            "widx": pr["widx"][c],
            "wloc": pr["wloc"][c],
            "wsc": pr["wsc"][c],
            "vfT": pr["vfT"][c],
            "cw_bf": cw_bf,
            "ww_bf": ww_bf,
            "w2_bf": w2_bf,
            "lw_bf": lw_bf,
            "lb_col": lb_col,
            "ident": ident,
            "sidx": pr["sidx"][c],
            "iota_bf": iota_bf,
        })
    res = run_bass_kernel_spmd(nc, in_maps, list(range(NC)))
    scores = np.empty(B, np.float32)
    for c in range(NC):
        w = res.results[c]["scores_w"]           # [128, 8]
        sc = np.asarray(w, np.float32).T.ravel()  # sc[position]
        scores[pr["outperm"][c]] = sc
    return scores


kernel.run_traced = None  # set by test harness if needed


# revision 29
# speedup vs baseline: 3.3319x; 1.0417x over previous
"""GCMC (gnn_message_passing) Trainium2 Bass kernel, 8-core SPMD, v2.

Strategy (hardcoded for the nn_GCMC_40870908789353 shapes):
- Score-pair sharding: core c owns pairs [1024c, 1024(c+1)). Its 2048 node
  slots (1024 user + 1024 item, duplicates allowed) are the only rows for
  which agg/x2 are computed, so no collective is needed anywhere.
- Gathers use batched dma_gather (SWDGE, int16 idxs) instead of per-chunk
  indirect DMAs: ~30 instructions/core instead of ~850. Tables are split
  into <=32767-row regions (id_emb 3, word_table 4) to fit int16 indices.
- Transposed dataflow: segment-sum one-hot matmuls run as lhsT=payload,
  rhs=one-hot so PSUM holds agg^T / t_feat^T ([dim, slot]); every later
  matmul chains without a single PE transpose. lin_b rides the ACT bias
  port, x1@W and f@w2 accumulate into the same PSUM tile.
- Edge payload rows are L2-normalized on the fly (square/reduce/rsqrt on
  DVE) which folds F.normalize into the gather and kills the full-table
  normalize pass; the scale-mult also casts the payload to bf16.
- Word payload rows are pre-scaled by 1/deg(item slot) (host metadata), so
  the PSUM directly accumulates the mean.
"""
import sys
for p in ("/opt/trn_rl_repo", "/root/.axon_site/_ro/trn_rl_repo"):
    if p not in sys.path:
        sys.path.insert(0, p)
import numpy as np
import ml_dtypes

NC = 8
NUM_USER = 50000
NUM_ITEM = 20000
NNODE = 70000
VOCAB = 100000
DIM = 64
WDIM = 128
B = 8192
BPC = 1024          # pairs per core
NSLOT = 2048        # node slots per core (1024 user + 1024 item)
NT = 16             # node slot tiles (128 slots, for the x-tail)
IT = 8              # item slot tiles (128 slots, for the f-pipeline)
ET = 32             # edge dst groups (64 slots each)
WT = 16             # word dst groups (64 slots each)
SLOTW = 64          # one-hot width per dst group
E_REG_BOUNDS = (0, 25000, 50000, 70000)   # aligned to user/item boundary
NREG_E = 3
REG_W = 25000       # word_table region rows (4 regions)
NREG_W = 4
EB = 32             # edge chunks per dma_gather batch
WB = 24             # word chunks per dma_gather batch
SLOPE = 0.01

_CACHE = {}

bf16 = ml_dtypes.bfloat16


# ---------------------------------------------------------------- CPU prep

def _ragged_gather(starts, lens):
    """positions [starts[i], starts[i]+lens[i]) concatenated."""
    tot = int(lens.sum())
    if tot == 0:
        return np.zeros(0, np.int64)
    cum = np.cumsum(lens) - lens
    return np.repeat(starts - cum, lens) + np.arange(tot)


def _build_stream(slot_rep, val_rep, region_rep, n_tiles, n_reg, extra=None):
    """Per-core stream fill given the instance list (slot, table-local idx,
    region). Returns dict with per-(region,tile) counts and a fill closure.
    """
    key = region_rep * n_tiles + (slot_rep >> 7)
    order = np.argsort(key, kind="stable")
    return order, key[order]


class _Sched:
    """Unified SPMD schedule for one gather family."""

    def __init__(self, cnt, n_tiles, n_reg, batch):
        # cnt: [NC, n_reg, n_tiles] instance counts
        nch = np.ceil(cnt / 128.0).astype(np.int64).max(axis=0)  # [n_reg,n_tiles]
        # every tile needs >=1 chunk overall so start/stop exist
        tile_tot = nch.sum(axis=0)
        for t in range(n_tiles):
            if tile_tot[t] == 0:
                nch[0][t] = 1
        self.nch = nch
        self.n_tiles = n_tiles
        self.n_reg = n_reg
        # global chunk order: region-major, tile-minor
        tiles = []
        regions = []
        for r in range(n_reg):
            for t in range(n_tiles):
                tiles += [t] * int(nch[r][t])
                regions += [r] * int(nch[r][t])
        self.tile_of = np.array(tiles, np.int64)
        self.region_of = np.array(regions, np.int64)
        self.NCH = len(tiles)
        self.S = self.NCH * 128
        # start/stop flags per chunk at (region, tile) GROUP granularity: each
        # group is one PSUM accumulation (own bank) closed within its region.
        self.is_first = []
        self.is_last = []
        for ch in range(len(tiles)):
            r, t = regions[ch], tiles[ch]
            self.is_first.append(ch == 0 or (regions[ch - 1], tiles[ch - 1]) != (r, t))
            self.is_last.append(ch == len(tiles) - 1
                                or (regions[ch + 1], tiles[ch + 1]) != (r, t))
        # group (r,t) -> starting chunk
        self.group_ch0 = np.zeros((n_reg, n_tiles), np.int64)
        ch = 0
        for r in range(n_reg):
            for t in range(n_tiles):
                self.group_ch0[r][t] = ch
                ch += int(nch[r][t])
        # batches: split each region's chunk range into <= batch chunks
        self.batches = []  # (region, ch0, nchunks)
        for r in range(n_reg):
            r0 = int(self.group_ch0[r][0])
            r1 = int(self.group_ch0[r + 1][0]) if r + 1 < n_reg else self.NCH
            ch = r0
            while ch < r1:
                nb = min(batch, r1 - ch)
                # Never cut a batch right after a group's first chunk: a
                # continuing group would then open with a 64-partition single
                # whose start=True clears has_written only for partitions
                # 0-63, leaving the pairs' q11 half to accumulate onto the
                # PSUM slot's stale contents.
                if ch + nb < r1:
                    last = ch + nb - 1
                    if self.is_first[last] and not self.is_last[last]:
                        nb -= 1
                self.batches.append((r, ch, nb))
                ch += nb

    def key(self):
        return (self.n_tiles, self.n_reg) + tuple(self.nch.ravel().tolist())


def _fill_stream(sched, slot_rep, loc_val, region_rep, scale=None):
    """Place instances into the padded stream. Returns (idx_stream int16,
    loc_stream bf16, scale_stream bf16 or None)."""
    n_tiles = sched.n_tiles
    key = region_rep * n_tiles + (slot_rep >> 6)
    order = np.argsort(key, kind="stable")
    skey = key[order]
    gcnt = np.bincount(skey, minlength=sched.n_reg * n_tiles)
    # position of each sorted instance: group base*128 + within-group offset
    ch0 = sched.group_ch0.ravel()
    base = np.repeat(ch0 * 128, gcnt)
    within = np.arange(len(order)) - np.repeat(np.cumsum(gcnt) - gcnt, gcnt)
    pos = base + within
    idx_stream = np.zeros(sched.S, np.int16)
    idx_stream[pos] = loc_val[order].astype(np.int16)
    loc_stream = np.full(sched.S, -1.0, bf16)
    loc_stream[pos] = (slot_rep[order] & 63).astype(bf16)
    sc_stream = None
    if scale is not None:
        sc_stream = np.zeros(sched.S, bf16)
        sc_stream[pos] = scale[order].astype(bf16)
    return idx_stream, loc_stream, sc_stream


def _wrap_idx(idx_stream):
    """[S] int16 -> [128, S/16] wrapped+replicated layout."""
    S = idx_stream.shape[0]
    base = idx_stream.reshape(S // 16, 16).T  # [16, S/16]
    return np.ascontiguousarray(np.tile(base, (8, 1)))


def _per_chunk(stream):
    """[S] -> [128, NCH]: position i=(ch*128+p) -> [p, ch]."""
    NCH = stream.shape[0] // 128
    return np.ascontiguousarray(stream.reshape(NCH, 128).T)


def _prep(inputs):
    edge_index = np.asarray(inputs["edge_index"])
    words_tensor = np.asarray(inputs["words_tensor"])
    user_nodes = np.asarray(inputs["user_nodes"]).astype(np.int64)
    item_nodes = np.asarray(inputs["item_nodes"]).astype(np.int64)

    src = edge_index[0].astype(np.int64)
    dst = edge_index[1].astype(np.int64)
    witem = words_tensor[0].astype(np.int64)
    wword = words_tensor[1].astype(np.int64)

    eorder = np.argsort(dst, kind="stable")
    sdst = dst[eorder]
    ssrc = src[eorder]
    worder = np.argsort(witem, kind="stable")
    switem_srt = witem[worder]
    swword = wword[worder]

    deg = np.bincount(dst, minlength=NNODE)
    wc_item = np.bincount(witem, minlength=NUM_ITEM)

    def snake_pos(n):
        i = np.arange(n)
        rnd, lane = divmod(i, 16)
        g = np.where(rnd % 2 == 0, lane, 15 - lane)
        return g * 64 + rnd

    # cluster pairs by item: each item's aggregation lands on one core
    gorder = np.argsort(item_nodes, kind="stable")

    e_data, w_data = [], []
    outperm = np.zeros((NC, BPC), np.int64)
    sidx = np.zeros((NC, 128, BPC // 16), np.int16)
    cnt_e = np.zeros((NC, NREG_E, ET), np.int64)
    cnt_w = np.zeros((NC, NREG_W, WT), np.int64)
    vfT = np.zeros((NC, WDIM, BPC), bf16)
    v_feat = np.asarray(inputs["v_feat"], np.float32)
    for c in range(NC):
        P = gorder[c * BPC:(c + 1) * BPC]
        users = user_nodes[P]
        items = item_nodes[P]
        # user position permutation (balance by degree, snake)
        order_u = np.argsort(-deg[users], kind="stable")
        pos_u = snake_pos(BPC)
        uperm = np.empty(BPC, np.int64)       # uperm[position] = pair rank in P
        uperm[pos_u] = order_u
        outperm[c] = P[uperm]
        # unique items -> balanced slot positions
        uit = np.unique(items)                # sorted node ids
        nu = len(uit)
        iid = uit - NUM_USER
        order_i = np.argsort(-(deg[uit] + wc_item[iid]), kind="stable")
        # rank r (in uit order) -> its balance order index, then snake position
        inv = np.empty(nu, np.int64)
        inv[order_i] = np.arange(nu)
        ipos_of_rank = snake_pos(nu)[inv]
        # final-score gather: position q -> item slot position
        islot_of_pair = ipos_of_rank[np.searchsorted(uit, items[uperm])]
        st16 = islot_of_pair.astype(np.int16)
        sidx[c] = np.ascontiguousarray(
            np.tile(st16.reshape(BPC // 16, 16).T, (8, 1)))

        # edge instances: user positions + unique-item slots
        nodes_e = np.concatenate([users[uperm], uit])
        slots_e = np.concatenate([np.arange(BPC), BPC + ipos_of_rank])
        st = np.searchsorted(sdst, nodes_e)
        en = np.searchsorted(sdst, nodes_e, side="right")
        lens = en - st
        slot_rep = np.repeat(slots_e, lens)
        src_rep = ssrc[_ragged_gather(st, lens)]
        reg_rep = np.searchsorted(np.array(E_REG_BOUNDS[1:-1]), src_rep,
                                  side="right")
        loc_rep = src_rep - np.array(E_REG_BOUNDS)[reg_rep]
        np.add.at(cnt_e[c], (reg_rep, slot_rep >> 6), 1)
        e_data.append((slot_rep, loc_rep, reg_rep))

        # word instances per unique item
        wst = np.searchsorted(switem_srt, iid)
        wen = np.searchsorted(switem_srt, iid, side="right")
        wlens = wen - wst
        wslot_rep = np.repeat(ipos_of_rank, wlens)
        word_rep = swword[_ragged_gather(wst, wlens)]
        wreg_rep = word_rep // REG_W
        wloc_rep = word_rep - wreg_rep * REG_W
        np.add.at(cnt_w[c], (wreg_rep, wslot_rep >> 6), 1)
        winv = (1.0 / np.maximum(wlens, 1)).astype(np.float32)
        wscale_rep = np.repeat(winv, wlens)
        w_data.append((wslot_rep, wloc_rep, wreg_rep, wscale_rep))

        vf_pos = np.zeros((BPC, WDIM), np.float32)
        vf_pos[ipos_of_rank] = v_feat[iid]
        vfT[c] = vf_pos.T.astype(bf16)

    es = _Sched(cnt_e, ET, NREG_E, EB)
    ws = _Sched(cnt_w, WT, NREG_W, WB)

    eidx = np.zeros((NC, 128, es.S // 16), np.int16)
    eloc = np.zeros((NC, 128, es.NCH), bf16)
    widx = np.zeros((NC, 128, ws.S // 16), np.int16)
    wloc = np.zeros((NC, 128, ws.NCH), bf16)
    wsc = np.zeros((NC, 128, ws.NCH), bf16)
    for c in range(NC):
        slot_rep, loc_rep, reg_rep = e_data[c]
        i_s, l_s, _ = _fill_stream(es, slot_rep, loc_rep, reg_rep)
        eidx[c] = _wrap_idx(i_s)
        eloc[c] = _per_chunk(l_s)
        wslot_rep, wloc_rep, wreg_rep, wscale_rep = w_data[c]
        i_s, l_s, s_s = _fill_stream(ws, wslot_rep, wloc_rep, wreg_rep,
                                     scale=wscale_rep)
        widx[c] = _wrap_idx(i_s)
        wloc[c] = _per_chunk(l_s)
        wsc[c] = _per_chunk(s_s)

    return dict(es=es, ws=ws, eidx=eidx, eloc=eloc,
                widx=widx, wloc=wloc, wsc=wsc, vfT=vfT,
                sidx=sidx, outperm=outperm)


# ------------------------------------------------------------- bass program

def _build_program(es, ws):
    from concourse import bass, bacc, mybir
    import concourse.tile as tile
    dt = mybir.dt

    nc = bacc.Bacc(None, target_bir_lowering=False, num_swdge_queues=4)
    f32 = dt.float32
    bf = dt.bfloat16

    id_in = nc.dram_tensor("id_emb", [NNODE, DIM], f32, kind="ExternalInput")
    wt_in = nc.dram_tensor("wt_bf", [VOCAB, WDIM], bf, kind="ExternalInput")
    eidx_in = nc.dram_tensor("eidx", [128, es.S // 16], dt.int16, kind="ExternalInput")
    eloc_in = nc.dram_tensor("eloc", [128, es.NCH], bf, kind="ExternalInput")
    widx_in = nc.dram_tensor("widx", [128, ws.S // 16], dt.int16, kind="ExternalInput")
    wloc_in = nc.dram_tensor("wloc", [128, ws.NCH], bf, kind="ExternalInput")
    wsc_in = nc.dram_tensor("wsc", [128, ws.NCH], bf, kind="ExternalInput")
    vfT_in = nc.dram_tensor("vfT", [WDIM, BPC], bf, kind="ExternalInput")
    cw_in = nc.dram_tensor("cw_bf", [DIM, DIM], bf, kind="ExternalInput")
    ww_in = nc.dram_tensor("ww_bf", [DIM, DIM], bf, kind="ExternalInput")
    w2_in = nc.dram_tensor("w2_bf", [DIM, DIM], bf, kind="ExternalInput")
    lw_in = nc.dram_tensor("lw_bf", [2 * WDIM, DIM], bf, kind="ExternalInput")
    lb_in = nc.dram_tensor("lb_col", [DIM, 1], f32, kind="ExternalInput")
    ident_in = nc.dram_tensor("ident", [128, 128], f32, kind="ExternalInput")
    sidx_in = nc.dram_tensor("sidx", [128, BPC // 16], dt.int16, kind="ExternalInput")
    iota_in = nc.dram_tensor("iota_bf", [128, 128], bf, kind="ExternalInput")
    out = nc.dram_tensor("scores_w", [128, 8], f32, kind="ExternalOutput")
    x2i_dram = nc.dram_tensor("x2i", [BPC, DIM], f32)
    import os
    DBG = os.environ.get("KDBG") == "1"
    if DBG:
        dbg_agg = nc.dram_tensor("dbg_agg", [DIM, NT * 128], f32, kind="ExternalOutput")
        dbg_tf = nc.dram_tensor("dbg_tf", [WDIM, IT * 128], f32, kind="ExternalOutput")
        dbg_x2 = nc.dram_tensor("dbg_x2", [DIM, NT * 128], f32, kind="ExternalOutput")
        dbg_ipay = nc.dram_tensor("dbg_ipay", [128, IT * DIM], f32, kind="ExternalOutput")

    id_regions = [(E_REG_BOUNDS[i], E_REG_BOUNDS[i + 1]) for i in range(3)]
    wt_regions = [(r * REG_W, (r + 1) * REG_W) for r in range(NREG_W)]

    with tile.TileContext(nc) as tc:
        with tc.tile_pool(name="const", bufs=1) as cpool, \
             tc.tile_pool(name="persist", bufs=1) as pp, \
             tc.tile_pool(name="ewp", bufs=4) as ewp, \
             tc.tile_pool(name="wwp", bufs=4) as wwp, \
             tc.tile_pool(name="mid", bufs=2) as midp, \
             tc.tile_pool(name="xp", bufs=2) as xp, \
             tc.tile_pool(name="psw", bufs=2, space="PSUM") as psw, \
             tc.tile_pool(name="pse", bufs=2, space="PSUM") as pse, \
             tc.tile_pool(name="psm", bufs=2, space="PSUM") as psm:

            iota = cpool.tile([128, 128], bf)
            cw = cpool.tile([DIM, DIM], bf)
            ww = cpool.tile([DIM, DIM], bf)
            w2 = cpool.tile([DIM, DIM], bf)
            lw = cpool.tile([128, 2 * DIM], bf)   # cols 0:64 = v-half, 64:128 = t-half
            lb = cpool.tile([DIM, 1], f32)
            ident = cpool.tile([128, 128], f32)
            sidx_sb = cpool.tile([128, BPC // 16], dt.int16)
            nc.sync.dma_start(out=iota[:], in_=iota_in[:])
            nc.sync.dma_start(out=cw[:], in_=cw_in[:])
            nc.sync.dma_start(out=ww[:], in_=ww_in[:])
            nc.sync.dma_start(out=w2[:], in_=w2_in[:])
            nc.sync.dma_start(out=lw[:, 0:DIM], in_=lw_in[0:128, :])
            nc.sync.dma_start(out=lw[:, DIM:2 * DIM], in_=lw_in[128:256, :])
            nc.sync.dma_start(out=lb[:], in_=lb_in[:])
            nc.sync.dma_start(out=ident[:], in_=ident_in[:])
            nc.sync.dma_start(out=sidx_sb[:], in_=sidx_in[:])

            eidx_sb = pp.tile([128, es.S // 16], dt.int16)
            eloc_sb = pp.tile([128, es.NCH], bf)
            widx_sb = pp.tile([128, ws.S // 16], dt.int16)
            wloc_sb = pp.tile([128, ws.NCH], bf)
            wsc_sb = pp.tile([128, ws.NCH], bf)
            vfT_sb = pp.tile([WDIM, BPC], bf)
            nc.sync.dma_start(out=eidx_sb[:], in_=eidx_in[:])
            nc.sync.dma_start(out=eloc_sb[:], in_=eloc_in[:])
            nc.sync.dma_start(out=widx_sb[:], in_=widx_in[:])
            nc.sync.dma_start(out=wloc_sb[:], in_=wloc_in[:])
            nc.sync.dma_start(out=wsc_sb[:], in_=wsc_in[:])
            nc.sync.dma_start(out=vfT_sb[:], in_=vfT_in[:])

            tfT_sb = pp.tile([WDIM, IT * 128], bf)
            fT_sb = pp.tile([DIM, IT * 128], bf)
            x2T_sb = pp.tile([DIM, NT * 128], f32)
            tfsum_sb = pp.tile([WDIM, IT * 128], f32)
            agg_sb = pp.tile([DIM, NT * 128], f32)
            nc.vector.memset(tfsum_sb[:], 0.0)
            nc.vector.memset(agg_sb[:], 0.0)

            # ---- words: t_feat^T accumulation ----
            wps = None
            for wq, (r, ch0, nb) in enumerate(ws.batches):
                r0, r1 = wt_regions[r]
                wpay = wwp.tile([128, WB * WDIM], bf, tag="wpay")
                pay3 = wpay[:].rearrange("p (k d) -> p k d", d=WDIM)
                nc.gpsimd.dma_gather(
                    wpay[:, 0:nb * WDIM].rearrange("p (k d) -> p k d", d=WDIM),
                    wt_in[r0:r1, :],
                    widx_sb[:, ch0 * 8:(ch0 + nb) * 8],
                    nb * 128, nb * 128, WDIM, single_packet=False,
                    queue_num=wq % 4)
                wpays = midp.tile([128, WB * WDIM], bf, tag="wpays")
                pays3 = wpays[:].rearrange("p (k d) -> p k d", d=WDIM)
                nc.vector.tensor_tensor(
                    out=pays3[:, 0:nb, :], in0=pay3[:, 0:nb, :],
                    in1=wsc_sb[:, ch0:ch0 + nb][:, :, None].to_broadcast(
                        [128, nb, WDIM]),
                    op=mybir.AluOpType.mult)
                woh = wwp.tile([128, WB * SLOTW], bf, tag="woh")
                oh3 = woh[:].rearrange("p (k d) -> p k d", d=SLOTW)
                nc.vector.tensor_tensor(
                    out=oh3[:, 0:nb, :],
                    in0=wloc_sb[:, ch0:ch0 + nb][:, :, None].to_broadcast(
                        [128, nb, SLOTW]),
                    in1=iota[:][:, None, 0:SLOTW].to_broadcast([128, nb, SLOTW]),
                    op=mybir.AluOpType.is_equal)
                for k in range(nb):
                    ch = ch0 + k
                    t = int(ws.tile_of[ch])
                    if ws.is_first[ch]:
                        wps = psw.tile([WDIM, 512], f32, tag="wp")
                    nc.tensor.matmul(
                        out=wps[:, 0:SLOTW], lhsT=pays3[:, k, :], rhs=oh3[:, k, :],
                        start=ws.is_first[ch], stop=ws.is_last[ch])
                    if ws.is_last[ch]:
                        sl = tfsum_sb[:, t * SLOTW:(t + 1) * SLOTW]
                        nc.vector.tensor_tensor(out=sl, in0=sl,
                                                in1=wps[:, 0:SLOTW],
                                                op=mybir.AluOpType.add)

            for t in range(IT):
                nc.scalar.activation(
                    tfT_sb[:, t * 128:(t + 1) * 128],
                    tfsum_sb[:, t * 128:(t + 1) * 128],
                    mybir.ActivationFunctionType.Copy)

            # ---- f^T = lrelu(lw^T cat^T + lb); fh feeds item-tile x2 ----
            for t in range(IT):
                fp = psm.tile([DIM, 512], f32, tag="mm")
                nc.tensor.matmul(out=fp[:, 0:128], lhsT=lw[:, 0:DIM],
                                 rhs=vfT_sb[:, t * 128:(t + 1) * 128],
                                 start=True, stop=False)
                nc.tensor.matmul(out=fp[:, 0:128], lhsT=lw[:, DIM:2 * DIM],
                                 rhs=tfT_sb[:, t * 128:(t + 1) * 128],
                                 start=False, stop=True)
                nc.scalar.activation(
                    fT_sb[:, t * 128:(t + 1) * 128], fp[:, 0:128],
                    mybir.ActivationFunctionType.Lrelu,
                    bias=lb[:], alpha=SLOPE)

            # ---- edges: agg^T accumulation with on-the-fly normalize ----
            es_has_pair = set()
            for (_r, _c0, _nb) in es.batches:
                _k = 0
                while _k < _nb:
                    _ch = _c0 + _k
                    if (_k + 1 < _nb) and not es.is_first[_ch + 1]:
                        es_has_pair.add((int(es.region_of[_ch]),
                                         int(es.tile_of[_ch])))
                        _k += 2
                    else:
                        _k += 1
            if True:
              eps = None
              for eq, (r, ch0, nb) in enumerate(es.batches):
                r0, r1 = id_regions[r]
                epay = ewp.tile([128, EB * DIM], f32, tag="epay")
                pay3 = epay[:].rearrange("p (k d) -> p k d", d=DIM)
                nc.gpsimd.dma_gather(
                    epay[:, 0:nb * DIM].rearrange("p (k d) -> p k d", d=DIM),
                    id_in[r0:r1, :],
                    eidx_sb[:, ch0 * 8:(ch0 + nb) * 8],
                    nb * 128, nb * 128, DIM, single_packet=False,
                    queue_num=eq % 4)
                esq = midp.tile([128, EB * DIM], f32, tag="esq")
                sq3 = esq[:].rearrange("p (k d) -> p k d", d=DIM)
                nc.vector.tensor_tensor(out=sq3[:, 0:nb, :], in0=pay3[:, 0:nb, :],
                                        in1=pay3[:, 0:nb, :],
                                        op=mybir.AluOpType.mult)
                ss = ewp.tile([128, EB], f32, tag="ess")
                nc.vector.reduce_sum(out=ss[:, 0:nb], in_=sq3[:, 0:nb, :],
                                     axis=mybir.AxisListType.X)
                nc.scalar.sqrt(ss[:, 0:nb], ss[:, 0:nb])
                nc.vector.reciprocal(ss[:, 0:nb], ss[:, 0:nb])
                epayb = ewp.tile([128, EB * DIM], bf, tag="epayb")
                payb3 = epayb[:].rearrange("p (k d) -> p k d", d=DIM)
                nc.vector.tensor_tensor(
                    out=payb3[:, 0:nb, :], in0=pay3[:, 0:nb, :],
                    in1=ss[:, 0:nb][:, :, None].to_broadcast([128, nb, DIM]),
                    op=mybir.AluOpType.mult)
                eoh = ewp.tile([128, EB * SLOTW], bf, tag="eoh")
                oh3 = eoh[:].rearrange("p (k d) -> p k d", d=SLOTW)
                nc.vector.tensor_tensor(
                    out=oh3[:, 0:nb, :],
                    in0=eloc_sb[:, ch0:ch0 + nb][:, :, None].to_broadcast(
                        [128, nb, SLOTW]),
                    in1=iota[:][:, None, 0:SLOTW].to_broadcast([128, nb, SLOTW]),
                    op=mybir.AluOpType.is_equal)
                k = 0
                while k < nb:
                    ch = ch0 + k
                    t = int(es.tile_of[ch])
                    if es.is_first[ch]:
                        eps = pse.tile([128, 512], f32, tag="ep")
                    pair = (k + 1 < nb) and not es.is_first[ch + 1]
                    if pair:
                        stop = es.is_last[ch + 1]
                        nc.tensor.matmul(
                            out=eps[:, 0:128],
                            lhsT=epayb[:, k * DIM:(k + 2) * DIM],
                            rhs=eoh[:, k * SLOTW:(k + 2) * SLOTW],
                            start=es.is_first[ch], stop=stop)
                        k += 2
                    else:
                        stop = es.is_last[ch]
                        nc.tensor.matmul(
                            out=eps[0:DIM, 0:SLOTW],
                            lhsT=epayb[:, k * DIM:(k + 1) * DIM],
                            rhs=eoh[:, k * SLOTW:(k + 1) * SLOTW],
                            start=es.is_first[ch], stop=stop)
                        k += 1
                    if stop:
                        g = (int(es.region_of[ch]), t)
                        sl = agg_sb[:, t * SLOTW:(t + 1) * SLOTW]
                        nc.vector.tensor_tensor(out=sl, in0=sl,
                                                in1=eps[0:DIM, 0:SLOTW],
                                                op=mybir.AluOpType.add)
                        if g in es_has_pair:
                            nc.vector.tensor_tensor(
                                out=sl, in0=sl,
                                in1=eps[DIM:128, SLOTW:128],
                                op=mybir.AluOpType.add)

              # ---- node tail: x2^T = lrelu(ww^T x1^T (+ w2^T f^T)) ----
              for t in range(NT):
                aggT = xp.tile([DIM, 128], bf, tag="aggT")
                nc.scalar.activation(aggT[:], agg_sb[:, t * 128:(t + 1) * 128],
                                     mybir.ActivationFunctionType.Copy)
                x1p = psm.tile([DIM, 512], f32, tag="mm")
                nc.tensor.matmul(out=x1p[:, 0:128], lhsT=cw[:], rhs=aggT[:],
                                 start=True, stop=True)
                x1T = xp.tile([DIM, 128], bf, tag="x1T")
                nc.scalar.activation(x1T[:], x1p[:, 0:128],
                                     mybir.ActivationFunctionType.Lrelu,
                                     alpha=SLOPE)
                x2p = psm.tile([DIM, 512], f32, tag="mm")
                nc.tensor.matmul(out=x2p[:, 0:128], lhsT=ww[:], rhs=x1T[:],
                                 start=True, stop=(t < IT))
                if t >= IT:
                    ti = t - IT
                    nc.tensor.matmul(out=x2p[:, 0:128], lhsT=w2[:],
                                     rhs=fT_sb[:, ti * 128:(ti + 1) * 128],
                                     start=False, stop=True)
                nc.scalar.activation(x2T_sb[:, t * 128:(t + 1) * 128], x2p[:, 0:128],
                                     mybir.ActivationFunctionType.Lrelu,
                                     alpha=SLOPE)

            # ---- scores: transpose x2^T tiles to rows, route item rows ----
            x2r_u = pp.tile([128, IT * DIM], f32)
            x2r_i = pp.tile([128, IT * DIM], f32)
            for t in range(IT):
                ps_t = psm.tile([128, 512], f32, tag="tr")
                nc.tensor.transpose(out=ps_t[:, 0:DIM],
                                    in_=x2T_sb[:, t * 128:(t + 1) * 128],
                                    identity=ident[0:DIM, 0:DIM])
                nc.scalar.activation(x2r_u[:, t * DIM:(t + 1) * DIM], ps_t[:, 0:DIM],
                                     mybir.ActivationFunctionType.Copy)
            for t in range(IT):
                ps_t = psm.tile([128, 512], f32, tag="tr")
                nc.tensor.transpose(out=ps_t[:, 0:DIM],
                                    in_=x2T_sb[:, (IT + t) * 128:(IT + t + 1) * 128],
                                    identity=ident[0:DIM, 0:DIM])
                nc.scalar.activation(x2r_i[:, t * DIM:(t + 1) * DIM], ps_t[:, 0:DIM],
                                     mybir.ActivationFunctionType.Copy)
            nc.sync.dma_start(
                out=x2i_dram[:, :].rearrange("(t p) d -> p t d", p=128),
                in_=x2r_i[:].rearrange("p (t d) -> p t d", d=DIM))
            ipay = pp.tile([128, IT * DIM], f32)
            nc.gpsimd.dma_gather(
                ipay[:].rearrange("p (k d) -> p k d", d=DIM),
                x2i_dram[:, :],
                sidx_sb[:],
                BPC, BPC, DIM, single_packet=False)
            prod = pp.tile([128, IT * DIM], f32)
            nc.vector.tensor_tensor(
                out=prod[:].rearrange("p (k d) -> p k d", d=DIM),
                in0=x2r_u[:].rearrange("p (k d) -> p k d", d=DIM),
                in1=ipay[:].rearrange("p (k d) -> p k d", d=DIM),
                op=mybir.AluOpType.mult)
            sc = pp.tile([128, 8], f32)
            nc.vector.reduce_sum(out=sc[:],
                                 in_=prod[:].rearrange("p (k d) -> p k d", d=DIM),
                                 axis=mybir.AxisListType.X)
            nc.sync.dma_start(out=out[:], in_=sc[:])
            if DBG:
                nc.sync.dma_start(out=dbg_agg[:], in_=agg_sb[:])
                nc.sync.dma_start(out=dbg_tf[:], in_=tfsum_sb[:])
                nc.sync.dma_start(out=dbg_x2[:], in_=x2T_sb[:])
                nc.sync.dma_start(out=dbg_ipay[:], in_=ipay[:])

    nc.finalize()
    return nc


# ------------------------------------------------------------------- kernel

def kernel(**inputs):
    from concourse.bass_utils import run_bass_kernel_spmd

    pr = _prep(inputs)
    es, ws = pr["es"], pr["ws"]
    key = es.key() + ws.key()
    if key not in _CACHE:
        _CACHE[key] = _build_program(es, ws)
    nc = _CACHE[key]

    iota_bf = np.broadcast_to(np.arange(128, dtype=bf16), (128, 128)).copy()
    ident = np.eye(128, dtype=np.float32)
    wt_bf = np.asarray(inputs["word_table"], np.float32).astype(bf16)
    lb_col = np.asarray(inputs["lin_b"], np.float32).reshape(DIM, 1).copy()
    cw_bf = np.asarray(inputs["conv_weight"], np.float32).astype(bf16)
    ww_bf = np.asarray(inputs["weight_W"], np.float32).astype(bf16)
    w2_bf = np.asarray(inputs["weight_2"], np.float32).astype(bf16)
    lw_bf = np.asarray(inputs["lin_w"], np.float32).astype(bf16)
    id_emb = np.ascontiguousarray(np.asarray(inputs["id_embedding"], np.float32))

    in_maps = []
    for c in range(NC):
        in_maps.append({
            "id_emb": id_emb,
            "wt_bf": wt_bf,
            "eidx": pr["eidx"][c],
            "eloc": pr["eloc"][c],
            "widx": pr["widx"][c],
            "wloc": pr["wloc"][c],
            "wsc": pr["wsc"][c],
            "vfT": pr["vfT"][c],
            "cw_bf": cw_bf,
            "ww_bf": ww_bf,
            "w2_bf": w2_bf,
            "lw_bf": lw_bf,
            "lb_col": lb_col,
            "ident": ident,
            "sidx": pr["sidx"][c],
            "iota_bf": iota_bf,
        })
    res = run_bass_kernel_spmd(nc, in_maps, list(range(NC)))
    scores = np.empty(B, np.float32)
    for c in range(NC):
        w = res.results[c]["scores_w"]           # [128, 8]
        sc = np.asarray(w, np.float32).T.ravel()  # sc[position]
        scores[pr["outperm"][c]] = sc
    return scores


kernel.run_traced = None  # set by test harness if needed
